# revision 1
# baseline (speedup 1.0000x reference)
"""CABlock cross-attention kernel for 8 TRN2 NeuronCores.

Sharding: 8 cores = 4 batches x 2 query-halves. Each core computes a fully
independent output slice out[b, h*2048:(h+1)*2048, :] -- no collectives.
"""

import sys

import numpy as np

try:
    import concourse.bass as bass  # noqa: F401
except ImportError:
    sys.path.insert(0, "/opt/trn_rl_repo")
    import concourse.bass as bass

import ml_dtypes
import concourse.mybir as mybir
import concourse.tile as tile
from concourse.bass_utils import run_bass_kernel_spmd
from concourse.masks import make_identity

F32 = mybir.dt.float32
BF16 = mybir.dt.bfloat16
BF = ml_dtypes.bfloat16

# per-core problem dims
NQ = 2048   # query rows per core (16 tiles of 128)
M = 1024    # context rows (8 tiles of 128)
C = 256     # model dim (2 chunks of 128)
INNER = 512  # heads*dim_head (4 chunks of 128)
H = 8       # heads
DH = 64     # dim_head
NQT = NQ // 128   # 16
MT = M // 128     # 8
CC = C // 128     # 2
IC = INNER // 128  # 4
EPS = 1e-5

_CACHED_NC = None


def _split_multiwaits(nc):
    """walrus allows only one sem-wait per ISA instruction; move extra waits
    onto same-engine NoOps inserted immediately before the instruction."""
    cnt = 0
    for f in nc.m.functions:
        for b in f.blocks:
            out = []
            for inst in b.instructions:
                si = inst.sync_info
                if si is not None and si.on_wait and len(si.on_wait) > 1:
                    waits = list(si.on_wait)
                    for w in waits[:-1]:
                        cnt += 1
                        nop = mybir.InstNoOp(
                            name=f"WSPLIT-{cnt}",
                            ins=[], outs=[],
                            engine=inst.engine,
                            sync_info=mybir.SyncInfo(on_wait=[w], on_update=[]),
                            bass_nofuse=True,
                        )
                        out.append(nop)
                    inst.sync_info = mybir.SyncInfo(
                        on_wait=[waits[-1]], on_update=list(si.on_update)
                    )
                out.append(inst)
            b.instructions = out
    return nc


def _build_nc():
    nc = bass.Bass()
    x_ext = nc.declare_dram_parameter("xn", [NQ, C], F32, isOutput=False)
    y_ext = nc.declare_dram_parameter("yn", [M, C], F32, isOutput=False)
    wq_ext = nc.declare_dram_parameter("wq", [C, INNER], BF16, isOutput=False)
    wk_ext = nc.declare_dram_parameter("wk", [C, INNER], BF16, isOutput=False)
    wv_ext = nc.declare_dram_parameter("wv", [C, INNER], BF16, isOutput=False)
    wo_ext = nc.declare_dram_parameter("wo", [INNER, C], BF16, isOutput=False)
    out_ext = nc.declare_dram_parameter("out", [NQ, C], F32, isOutput=True)

    with tile.TileContext(nc) as tc:
        with (
            tc.tile_pool(name="singles", bufs=1) as singles,
            tc.tile_pool(name="big", bufs=1) as big,
            tc.tile_pool(name="probs", bufs=4) as probs_pool,
            tc.tile_pool(name="stats", bufs=4) as stats,
            tc.tile_pool(name="ps_big", bufs=2, space="PSUM") as ps_big,
            tc.tile_pool(name="ps_small", bufs=4, space="PSUM") as ps_small,
        ):
            ident = singles.tile([128, 128], F32)
            make_identity(nc, ident)
            ident_bf = singles.tile([128, 128], BF16)
            make_identity(nc, ident_bf)
            eps_t = singles.tile([128, 1], F32)
            nc.vector.memset(eps_t, EPS)

            # weights
            wq_sb = singles.tile([128, CC, INNER], BF16)
            nc.gpsimd.dma_start(wq_sb, wq_ext.rearrange("(kc p) i -> p kc i", p=128))
            wk_sb = singles.tile([128, CC, INNER], BF16)
            nc.gpsimd.dma_start(wk_sb, wk_ext.rearrange("(kc p) i -> p kc i", p=128))
            wv_sb = singles.tile([128, CC, INNER], BF16)
            nc.gpsimd.dma_start(wv_sb, wv_ext.rearrange("(kc p) i -> p kc i", p=128))
            wo_sb = singles.tile([128, IC, C], BF16)
            nc.gpsimd.dma_start(wo_sb, wo_ext.rearrange("(ic p) c -> p ic c", p=128))

            # PE primers: each PE instruction may carry only ONE sem wait, so
            # walk PE's observed vector clock over each foreign producer (Pool
            # for identities, the SWDGE queue for weights) one step at a time.
            prm = ps_small.tile([128, 512], F32, tag="ps_sm", name="prm1")
            nc.tensor.transpose(prm[:, :128], ident, ident)
            prm2 = ps_small.tile([128, 512], BF16, tag="ps_sm", name="prm2")
            nc.tensor.transpose(prm2[:, :128], ident_bf, ident_bf)
            prm3 = ps_small.tile([128, 512], BF16, tag="ps_sm", name="prm3")
            nc.tensor.transpose(prm3[:, :128], wo_sb[:, 0, :128], ident_bf)

            # ---- load x, y (n-layout) ----
            x_raw = big.tile([128, NQT, C], F32, tag="s16")
            xv = x_ext.rearrange("(t p) c -> p t c", p=128)
            for t in range(NQT):
                nc.gpsimd.dma_start(x_raw[:, t, :], xv[:, t, :])
            y_raw = big.tile([128, MT, C], F32)
            yv = y_ext.rearrange("(t p) c -> p t c", p=128)
            for t in range(MT):
                nc.gpsimd.dma_start(y_raw[:, t, :], yv[:, t, :])

            # ---- layernorm in n-layout, f32 (separate output tiles) ----
            def layernorm(dst, src, ntiles):
                for t in range(ntiles):
                    st = stats.tile([128, 6], F32, tag="bn6")
                    nc.vector.bn_stats(out=st, in_=src[:, t, :])
                    mv = stats.tile([128, 2], F32, tag="mv")
                    nc.vector.bn_aggr(out=mv, in_=st)
                    rstd = stats.tile([128, 1], F32, tag="rstd")
                    nc.scalar.activation(
                        out=rstd, in_=mv[:, 1:2],
                        func=mybir.ActivationFunctionType.Sqrt,
                        bias=eps_t, scale=1.0,
                    )
                    nc.vector.reciprocal(out=rstd, in_=rstd)
                    nc.vector.tensor_scalar(
                        out=dst[:, t, :], in0=src[:, t, :],
                        scalar1=mv[:, 0:1], scalar2=rstd,
                        op0=mybir.AluOpType.subtract, op1=mybir.AluOpType.mult,
                    )

            y_sb = big.tile([128, MT, C], F32)
            layernorm(y_sb, y_raw, MT)
            x_sb = big.tile([128, NQT, C], F32)
            layernorm(x_sb, x_raw, NQT)

            # ---- PE-transpose xn, yn -> c-layout bf16 ----
            xnT = big.tile([128, CC, NQ], BF16)
            for t in range(NQT):
                for cc in range(CC):
                    pt = ps_small.tile([128, 512], F32, tag="ps_sm")
                    nc.tensor.transpose(pt[:, :128], x_sb[:, t, cc * 128:(cc + 1) * 128], ident)
                    nc.vector.tensor_copy(out=xnT[:, cc, t * 128:(t + 1) * 128], in_=pt[:, :128])
            ynT = big.tile([128, CC, M], BF16)
            for t in range(MT):
                for cc in range(CC):
                    pt = ps_small.tile([128, 512], F32, tag="ps_sm")
                    nc.tensor.transpose(pt[:, :128], y_sb[:, t, cc * 128:(cc + 1) * 128], ident)
                    nc.vector.tensor_copy(out=ynT[:, cc, t * 128:(t + 1) * 128], in_=pt[:, :128])

            # ---- projections (bf16) ----
            # qT[inner, nq]
            qt = big.tile([128, IC, NQ], BF16)
            for ic in range(IC):
                for nqc in range(NQ // 512):
                    pq = ps_small.tile([128, 512], F32, tag="ps_sm")
                    for kc in range(CC):
                        nc.tensor.matmul(
                            pq, lhsT=wq_sb[:, kc, ic * 128:(ic + 1) * 128],
                            rhs=xnT[:, kc, nqc * 512:(nqc + 1) * 512],
                            start=(kc == 0), stop=(kc == CC - 1),
                        )
                    nc.vector.tensor_copy(out=qt[:, ic, nqc * 512:(nqc + 1) * 512], in_=pq)
            # kT[inner, m]
            kt = big.tile([128, IC, M], BF16)
            for ic in range(IC):
                for mc in range(M // 512):
                    pk = ps_small.tile([128, 512], F32, tag="ps_sm")
                    for kc in range(CC):
                        nc.tensor.matmul(
                            pk, lhsT=wk_sb[:, kc, ic * 128:(ic + 1) * 128],
                            rhs=ynT[:, kc, mc * 512:(mc + 1) * 512],
                            start=(kc == 0), stop=(kc == CC - 1),
                        )
                    nc.vector.tensor_copy(out=kt[:, ic, mc * 512:(mc + 1) * 512], in_=pk)
            # v[m, h, 65]  (col 64 = ones for row-sums)
            v_sb = big.tile([128, MT, H, DH + 1], BF16)
            nc.vector.memset(v_sb[:, :, :, DH:DH + 1], 1.0)
            for mt in range(MT):
                pv = ps_small.tile([128, 512], F32, tag="ps_sm")
                for kc in range(CC):
                    nc.tensor.matmul(
                        pv, lhsT=ynT[:, kc, mt * 128:(mt + 1) * 128],
                        rhs=wv_sb[:, kc, :],
                        start=(kc == 0), stop=(kc == CC - 1),
                    )
                nc.vector.tensor_copy(
                    out=v_sb[:, mt, :, 0:DH],
                    in_=pv.rearrange("p (h e) -> p h e", h=H),
                )
            # v primers: let PE observe every v tile's DVE tick before the
            # attention matmuls (else attn@v would need ACT + DVE waits).
            for mt in range(MT):
                pvp = ps_small.tile([128, 512], BF16, tag="ps_sm", name=f"vprm{mt}")
                nc.tensor.transpose(pvp[:65, :128], v_sb[:, mt, H - 1, :], ident_bf)

            # ---- attention, head pairs ----
            o_sb = big.tile([128, NQT, IC, 128], BF16, tag="s16")  # o[nq, inner]
            for hp in range(H // 2):
                for nqh in range(2):  # nq halves pipeline independently
                    pT = []
                    for hh in range(2):
                        pT.append(probs_pool.tile([128, MT, NQ // 2], BF16,
                                                  tag="probsT",
                                                  name=f"probsT_{hp}_{nqh}_{hh}"))
                    # scoresT + exp:  ET[nk, nq] = kT_h[:,nk_tile].T @ qT_h
                    for mt in range(MT):
                        pe = []
                        for hh in range(2):
                            p_e = ps_big.tile([128, 1024], F32, tag="escore")
                            lhsT = kt[hh * 64:(hh + 1) * 64, hp, mt * 128:(mt + 1) * 128]
                            for n2 in range(2):
                                nc.tensor.matmul(
                                    p_e[:, n2 * 512:(n2 + 1) * 512],
                                    lhsT=lhsT,
                                    rhs=qt[hh * 64:(hh + 1) * 64, hp,
                                           nqh * 1024 + n2 * 512:nqh * 1024 + (n2 + 1) * 512],
                                    start=True, stop=True,
                                )
                            pe.append(p_e)
                        for hh in range(2):
                            nc.scalar.activation(
                                out=pT[hh][:, mt, :],
                                in_=pe[hh],
                                func=mybir.ActivationFunctionType.Exp,
                            )
                    # attn@v: o[nq_tile, 65] = probsT[:,nq_tile].T @ v_aug
                    for lq in range(NQT // 2):
                        nqt = nqh * (NQT // 2) + lq
                        for hh in range(2):
                            h = hp * 2 + hh
                            po = ps_small.tile([128, 512], F32, tag="ps_sm")
                            for mt in range(MT):
                                nc.tensor.matmul(
                                    po[:, :DH + 1],
                                    lhsT=pT[hh][:, mt, lq * 128:(lq + 1) * 128],
                                    rhs=v_sb[:, mt, h, :],
                                    start=(mt == 0), stop=(mt == MT - 1),
                                )
                            rs = stats.tile([128, 1], F32, tag="rs")
                            nc.vector.reciprocal(out=rs, in_=po[:, DH:DH + 1])
                            nc.vector.tensor_scalar_mul(
                                out=o_sb[:, nqt, h // 2, (h % 2) * DH:(h % 2) * DH + DH],
                                in0=po[:, 0:DH], scalar1=rs,
                            )

            # ---- transpose o -> oT[inner, nq] ----
            oT = big.tile([128, IC, NQ], BF16)
            for ic in range(IC):
                for nqt in range(NQT):
                    pt = ps_small.tile([128, 512], BF16, tag="ps_sm")
                    nc.tensor.transpose(pt[:, :128], o_sb[:, nqt, ic, :], ident_bf)
                    nc.vector.tensor_copy(out=oT[:, ic, nqt * 128:(nqt + 1) * 128], in_=pt[:, :128])

            # ---- out-proj + residual ----
            for nqt in range(NQT):
                pf = ps_small.tile([128, 512], F32, tag="ps_sm")
                for ic in range(IC):
                    nc.tensor.matmul(
                        pf[:, :C],
                        lhsT=oT[:, ic, nqt * 128:(nqt + 1) * 128],
                        rhs=wo_sb[:, ic, :],
                        start=(ic == 0), stop=(ic == IC - 1),
                    )
                fin = stats.tile([128, C], F32, tag="fin")
                nc.vector.tensor_add(out=fin, in0=pf[:, :C], in1=x_sb[:, nqt, :])
                nc.gpsimd.dma_start(
                    out_ext.rearrange("(t p) c -> p t c", p=128)[:, nqt, :], fin
                )
    return _split_multiwaits(nc)


def _numpy_fallback(x, y, ln_x_g, ln_x_b, ln_y_g, ln_y_b, Wq, Wk, Wv, bv, Wo, bo):
    def ln(a, g, b):
        mu = a.mean(-1, keepdims=True)
        var = ((a - mu) ** 2).mean(-1, keepdims=True)
        return (a - mu) / np.sqrt(var + EPS) * g + b

    b_, c_ = x.shape[:2]
    xn = x.reshape(b_, c_, -1).swapaxes(1, 2)
    xn = ln(xn, ln_x_g, ln_x_b)
    yn = ln(y, ln_y_g, ln_y_b)
    q = xn @ Wq
    k = yn @ Wk
    v = yn @ Wv + bv

    def sh(t):
        B, N, _ = t.shape
        return t.reshape(B, N, H, DH).transpose(0, 2, 1, 3)

    q, k, v = sh(q), sh(k), sh(v)
    a = np.einsum("bhid,bhjd->bhij", q, k) * (DH ** -0.5)
    a = a - a.max(-1, keepdims=True)
    e = np.exp(a)
    a = e / e.sum(-1, keepdims=True)
    o = np.einsum("bhij,bhjd->bhid", a, v)
    o = o.transpose(0, 2, 1, 3).reshape(b_, -1, H * DH)
    return (xn + o @ Wo + bo).astype(np.float32)


def kernel(x, y, ln_x_g, ln_x_b, ln_y_g, ln_y_b, Wq, Wk, Wv, bv, Wo, bo, **kw):
    global _CACHED_NC
    x = np.asarray(x, np.float32)
    y = np.asarray(y, np.float32)
    if any(np.any(np.asarray(t)) for t in (ln_x_b, ln_y_b, bv, bo)):
        return _numpy_fallback(x, y, np.asarray(ln_x_g), np.asarray(ln_x_b),
                               np.asarray(ln_y_g), np.asarray(ln_y_b),
                               np.asarray(Wq), np.asarray(Wk), np.asarray(Wv),
                               np.asarray(bv), np.asarray(Wo), np.asarray(bo))

    wq = (np.asarray(ln_x_g, np.float32)[:, None] * np.asarray(Wq, np.float32)
          * (DH ** -0.5)).astype(BF)
    wk = (np.asarray(ln_y_g, np.float32)[:, None] * np.asarray(Wk, np.float32)).astype(BF)
    wv = (np.asarray(ln_y_g, np.float32)[:, None] * np.asarray(Wv, np.float32)).astype(BF)
    wo = np.asarray(Wo, np.float32).astype(BF)

    B = x.shape[0]
    N = x.shape[2] * x.shape[3]
    xf = x.reshape(B, C, N)
    in_maps = []
    for core in range(8):
        b, hf = core // 2, core % 2
        in_maps.append({
            "xn": np.ascontiguousarray(xf[b, :, hf * NQ:(hf + 1) * NQ].T),
            "yn": np.ascontiguousarray(y[b]),
            "wq": wq, "wk": wk, "wv": wv, "wo": wo,
        })

    if _CACHED_NC is None:
        _CACHED_NC = _build_nc()
    global _last_in_maps
    _last_in_maps = in_maps
    res = run_bass_kernel_spmd(_CACHED_NC, in_maps, list(range(8))).results

    out = np.empty((B, N, C), np.float32)
    for core in range(8):
        b, hf = core // 2, core % 2
        out[b, hf * NQ:(hf + 1) * NQ, :] = res[core]["out"]
    return out



# revision 2
# speedup vs baseline: 4.1050x; 4.1050x over previous
"""CABlock cross-attention kernel for 8 TRN2 NeuronCores.

Sharding: 8 cores = 4 batches x 2 query-halves; each core computes an
independent output slice, no collectives.

The axon tunnel (~50MB/s up, ~33MB/s down) dominates wall time, so the
host/device split minimizes bytes on the wire:
  - LayerNorm (incl. gains/biases) runs on host in f32; normalized
    activations ship as fp8 e4m3 in c-layout [C, N] (natural layout of x,
    so no transposes anywhere).
  - The device computes attention only and returns o@Wo in fp8 c-layout.
  - The residual add (exact f32 xn) happens on host; output biases
    (bv@Wo + bo) are folded in on host exactly.
  - Weights (bf16) are uploaded once and cached on device; the jitted
    executable is cached; the donated output buffer is recycled from the
    previous call, so steady-state traffic is 6MB up + 4MB down.
"""

import sys

import numpy as np

try:
    import concourse.bass as bass  # noqa: F401
except ImportError:
    sys.path.insert(0, "/opt/trn_rl_repo")
    import concourse.bass as bass

import ml_dtypes
import concourse.mybir as mybir
import concourse.tile as tile

F32 = mybir.dt.float32
BF16 = mybir.dt.bfloat16
F8 = mybir.dt.float8e4
BF = ml_dtypes.bfloat16
F8NP = ml_dtypes.float8_e4m3

# per-core problem dims
NQ = 2048   # query rows per core (16 tiles of 128)
M = 1024    # context rows (8 tiles of 128)
C = 256     # model dim (2 chunks of 128)
INNER = 512  # heads*dim_head (4 chunks of 128)
H = 8       # heads
DH = 64     # dim_head
NQT = NQ // 128   # 16
MT = M // 128     # 8
CC = C // 128     # 2
IC = INNER // 128  # 4
EPS = 1e-5

_CACHE = {}


def _split_multiwaits(nc):
    """walrus allows only one sem-wait per ISA instruction; move extra waits
    onto same-engine NoOps inserted immediately before the instruction."""
    cnt = 0
    for f in nc.m.functions:
        for b in f.blocks:
            out = []
            for inst in b.instructions:
                si = inst.sync_info
                if si is not None and si.on_wait and len(si.on_wait) > 1:
                    waits = list(si.on_wait)
                    for w in waits[:-1]:
                        cnt += 1
                        nop = mybir.InstNoOp(
                            name=f"WSPLIT-{cnt}",
                            ins=[], outs=[],
                            engine=inst.engine,
                            sync_info=mybir.SyncInfo(on_wait=[w], on_update=[]),
                            bass_nofuse=True,
                        )
                        out.append(nop)
                    inst.sync_info = mybir.SyncInfo(
                        on_wait=[waits[-1]], on_update=list(si.on_update)
                    )
                out.append(inst)
            b.instructions = out
    return nc


def _build_nc():
    from concourse.masks import make_identity

    nc = bass.Bass()
    x_ext = nc.declare_dram_parameter("xn", [C, NQ], F8, isOutput=False)
    y_ext = nc.declare_dram_parameter("yn", [C, M], F8, isOutput=False)
    wq_ext = nc.declare_dram_parameter("wq", [C, INNER], BF16, isOutput=False)
    wk_ext = nc.declare_dram_parameter("wk", [C, INNER], BF16, isOutput=False)
    wv_ext = nc.declare_dram_parameter("wv", [C, INNER], BF16, isOutput=False)
    wo_ext = nc.declare_dram_parameter("wo", [INNER, C], BF16, isOutput=False)
    out_ext = nc.declare_dram_parameter("out", [C, NQ], F8, isOutput=True)

    with tile.TileContext(nc) as tc:
        with (
            tc.tile_pool(name="singles", bufs=1) as singles,
            tc.tile_pool(name="big", bufs=1) as big,
            tc.tile_pool(name="probs", bufs=4) as probs_pool,
            tc.tile_pool(name="stats", bufs=4) as stats,
            tc.tile_pool(name="ps_big", bufs=2, space="PSUM") as ps_big,
            tc.tile_pool(name="ps_small", bufs=4, space="PSUM") as ps_small,
        ):
            ident_bf = singles.tile([128, 128], BF16)
            make_identity(nc, ident_bf)

            # weights (device-HBM resident across calls; cheap DMA here)
            wq_sb = singles.tile([128, CC, INNER], BF16)
            nc.gpsimd.dma_start(wq_sb, wq_ext.rearrange("(kc p) i -> p kc i", p=128))
            wk_sb = singles.tile([128, CC, INNER], BF16)
            nc.gpsimd.dma_start(wk_sb, wk_ext.rearrange("(kc p) i -> p kc i", p=128))
            wv_sb = singles.tile([128, CC, INNER], BF16)
            nc.gpsimd.dma_start(wv_sb, wv_ext.rearrange("(kc p) i -> p kc i", p=128))
            wo_sb = singles.tile([128, IC, C], BF16)
            nc.gpsimd.dma_start(wo_sb, wo_ext.rearrange("(ic p) c -> p ic c", p=128))

            # activations: host-LayerNormed, fp8, c-layout -- no transposes
            xnT = big.tile([128, CC, NQ], F8)
            nc.gpsimd.dma_start(xnT, x_ext.rearrange("(kc p) n -> p kc n", p=128))
            ynT = big.tile([128, CC, M], F8)
            nc.gpsimd.dma_start(ynT, y_ext.rearrange("(kc p) n -> p kc n", p=128))

            # ---- projections (PE reads fp8 activations directly) ----
            # qT[inner, nq]
            qt = big.tile([128, IC, NQ], BF16)
            for ic in range(IC):
                for nqc in range(NQ // 512):
                    pq = ps_small.tile([128, 512], F32, tag="ps_sm")
                    for kc in range(CC):
                        nc.tensor.matmul(
                            pq, lhsT=wq_sb[:, kc, ic * 128:(ic + 1) * 128],
                            rhs=xnT[:, kc, nqc * 512:(nqc + 1) * 512],
                            start=(kc == 0), stop=(kc == CC - 1),
                        )
                    nc.vector.tensor_copy(out=qt[:, ic, nqc * 512:(nqc + 1) * 512], in_=pq)
            # kT[inner, m]
            kt = big.tile([128, IC, M], BF16)
            for ic in range(IC):
                for mc in range(M // 512):
                    pk = ps_small.tile([128, 512], F32, tag="ps_sm")
                    for kc in range(CC):
                        nc.tensor.matmul(
                            pk, lhsT=wk_sb[:, kc, ic * 128:(ic + 1) * 128],
                            rhs=ynT[:, kc, mc * 512:(mc + 1) * 512],
                            start=(kc == 0), stop=(kc == CC - 1),
                        )
                    nc.vector.tensor_copy(out=kt[:, ic, mc * 512:(mc + 1) * 512], in_=pk)
            # v[m, h, 65]  (col 64 = ones for row-sums)
            v_sb = big.tile([128, MT, H, DH + 1], BF16)
            nc.vector.memset(v_sb[:, :, :, DH:DH + 1], 1.0)
            for mt in range(MT):
                pv = ps_small.tile([128, 512], F32, tag="ps_sm")
                for kc in range(CC):
                    nc.tensor.matmul(
                        pv, lhsT=ynT[:, kc, mt * 128:(mt + 1) * 128],
                        rhs=wv_sb[:, kc, :],
                        start=(kc == 0), stop=(kc == CC - 1),
                    )
                nc.vector.tensor_copy(
                    out=v_sb[:, mt, :, 0:DH],
                    in_=pv.rearrange("p (h e) -> p h e", h=H),
                )

            # ---- attention, head pairs ----
            o_sb = big.tile([128, NQT, IC, 128], BF16)  # o[nq, inner]
            for hp in range(H // 2):
                for nqh in range(2):  # nq halves pipeline independently
                    pT = []
                    for hh in range(2):
                        pT.append(probs_pool.tile([128, MT, NQ // 2], BF16,
                                                  tag="probsT",
                                                  name=f"probsT_{hp}_{nqh}_{hh}"))
                    # scoresT + exp:  ET[nk, nq] = kT_h[:,nk_tile].T @ qT_h
                    for mt in range(MT):
                        pe = []
                        for hh in range(2):
                            p_e = ps_big.tile([128, 1024], F32, tag="escore")
                            lhsT = kt[hh * 64:(hh + 1) * 64, hp, mt * 128:(mt + 1) * 128]
                            for n2 in range(2):
                                nc.tensor.matmul(
                                    p_e[:, n2 * 512:(n2 + 1) * 512],
                                    lhsT=lhsT,
                                    rhs=qt[hh * 64:(hh + 1) * 64, hp,
                                           nqh * 1024 + n2 * 512:nqh * 1024 + (n2 + 1) * 512],
                                    start=True, stop=True,
                                )
                            pe.append(p_e)
                        for hh in range(2):
                            nc.scalar.activation(
                                out=pT[hh][:, mt, :],
                                in_=pe[hh],
                                func=mybir.ActivationFunctionType.Exp,
                            )
                    # attn@v: o[nq_tile, 65] = probsT[:,nq_tile].T @ v_aug
                    for lq in range(NQT // 2):
                        nqt = nqh * (NQT // 2) + lq
                        for hh in range(2):
                            h = hp * 2 + hh
                            po = ps_small.tile([128, 512], F32, tag="ps_sm")
                            for mt in range(MT):
                                nc.tensor.matmul(
                                    po[:, :DH + 1],
                                    lhsT=pT[hh][:, mt, lq * 128:(lq + 1) * 128],
                                    rhs=v_sb[:, mt, h, :],
                                    start=(mt == 0), stop=(mt == MT - 1),
                                )
                            rs = stats.tile([128, 1], F32, tag="rs")
                            nc.vector.reciprocal(out=rs, in_=po[:, DH:DH + 1])
                            nc.vector.tensor_scalar_mul(
                                out=o_sb[:, nqt, h // 2, (h % 2) * DH:(h % 2) * DH + DH],
                                in0=po[:, 0:DH], scalar1=rs,
                            )

            # ---- transpose o -> oT[inner, nq] ----
            oT = big.tile([128, IC, NQ], BF16)
            for ic in range(IC):
                for nqt in range(NQT):
                    pt = ps_small.tile([128, 512], BF16, tag="ps_sm")
                    nc.tensor.transpose(pt[:, :128], o_sb[:, nqt, ic, :], ident_bf)
                    nc.vector.tensor_copy(out=oT[:, ic, nqt * 128:(nqt + 1) * 128], in_=pt[:, :128])

            # ---- out-proj, c-layout (lhsT/rhs swapped => no output transpose) ----
            out_sb = big.tile([128, CC, NQ], F8)
            for cc in range(CC):
                for nqc in range(NQ // 512):
                    pf = ps_small.tile([128, 512], F32, tag="ps_sm")
                    for ic in range(IC):
                        nc.tensor.matmul(
                            pf,
                            lhsT=wo_sb[:, ic, cc * 128:(cc + 1) * 128],
                            rhs=oT[:, ic, nqc * 512:(nqc + 1) * 512],
                            start=(ic == 0), stop=(ic == IC - 1),
                        )
                    nc.vector.tensor_copy(out=out_sb[:, cc, nqc * 512:(nqc + 1) * 512], in_=pf)
            nc.gpsimd.dma_start(out_ext.rearrange("(cc p) n -> p cc n", p=128), out_sb)
    return _split_multiwaits(nc)


def _get_runtime():
    """Build (once) the Bass module and a cached jitted SPMD executor."""
    if "sharded" in _CACHE:
        return _CACHE
    import jax
    from jax.sharding import Mesh, PartitionSpec, NamedSharding
    try:
        from jax.experimental.shard_map import shard_map
    except ImportError:
        from jax import shard_map
    from concourse.bass2jax import (
        _bass_exec_p, install_neuronx_cc_hook, partition_id_tensor,
    )

    nc = _build_nc()
    install_neuronx_cc_hook()

    partition_name = nc.partition_id_tensor.name if nc.partition_id_tensor else None
    in_names, out_names, out_avals = [], [], []
    for alloc in nc.m.functions[0].allocations:
        if not isinstance(alloc, mybir.MemoryLocationSet):
            continue
        name = alloc.memorylocations[0].name
        if alloc.kind == "ExternalInput":
            if name != partition_name:
                in_names.append(name)
        elif alloc.kind == "ExternalOutput":
            out_names.append(name)
            out_avals.append(jax.core.ShapedArray(
                tuple(alloc.tensor_shape), mybir.dt.np(alloc.dtype)))
    n_params = len(in_names)
    n_outs = len(out_avals)
    in_names_full = in_names + out_names + ([partition_name] if partition_name else [])

    def _body(*args):
        operands = list(args)
        if partition_name is not None:
            operands.append(partition_id_tensor())
        outs = _bass_exec_p.bind(
            *operands,
            out_avals=tuple(out_avals),
            in_names=tuple(in_names_full),
            out_names=tuple(out_names),
            lowering_input_output_aliases=(),
            sim_require_finite=True,
            sim_require_nnan=True,
            nc=nc,
        )
        return tuple(outs)

    devices = jax.devices()[:8]
    mesh = Mesh(np.asarray(devices), ("core",))
    shc = NamedSharding(mesh, PartitionSpec("core"))
    donate = tuple(range(n_params, n_params + n_outs))
    sharded = jax.jit(
        shard_map(_body, mesh=mesh,
                  in_specs=(PartitionSpec("core"),) * (n_params + n_outs),
                  out_specs=(PartitionSpec("core"),) * n_outs,
                  check_rep=False),
        donate_argnums=donate, keep_unused=True,
    )
    _CACHE.update(nc=nc, sharded=sharded, in_names=in_names, shc=shc,
                  jax=jax, device_put=jax.device_put)
    return _CACHE


def _layernorm_bcx(a, g, b):
    """LayerNorm over axis 1 of [B, C, N] (f32, exact, incl. gain/bias)."""
    mu = a.mean(axis=1, keepdims=True)
    ex2 = np.einsum("bcn,bcn->bn", a, a) / a.shape[1]
    var = ex2 - mu[:, 0, :] ** 2
    rstd = 1.0 / np.sqrt(var + EPS)
    out = (a - mu) * rstd[:, None, :]
    if not (g == 1.0).all():
        out *= g[:, None]
    if (b != 0.0).any():
        out += b[:, None]
    return out


def kernel(x, y, ln_x_g, ln_x_b, ln_y_g, ln_y_b, Wq, Wk, Wv, bv, Wo, bo, **kw):
    x = np.asarray(x, np.float32)
    y = np.asarray(y, np.float32)
    ln_x_g = np.asarray(ln_x_g, np.float32)
    ln_x_b = np.asarray(ln_x_b, np.float32)
    ln_y_g = np.asarray(ln_y_g, np.float32)
    ln_y_b = np.asarray(ln_y_b, np.float32)
    Wq_f = np.asarray(Wq, np.float32)
    Wk_f = np.asarray(Wk, np.float32)
    Wv_f = np.asarray(Wv, np.float32)
    Wo_f = np.asarray(Wo, np.float32)
    bv_f = np.asarray(bv, np.float32)
    bo_f = np.asarray(bo, np.float32)

    rt = _get_runtime()

    # ---- device-resident weights (re-uploaded only when they change) ----
    wkey = _CACHE.get("wkey")
    if wkey is None or not all(
        np.array_equal(a, b) for a, b in zip(wkey, (Wq_f, Wk_f, Wv_f, Wo_f))
    ):
        s = DH ** -0.5
        w_np = {
            "wq": np.concatenate([(Wq_f * s).astype(BF)] * 8, axis=0),
            "wk": np.concatenate([Wk_f.astype(BF)] * 8, axis=0),
            "wv": np.concatenate([Wv_f.astype(BF)] * 8, axis=0),
            "wo": np.concatenate([Wo_f.astype(BF)] * 8, axis=0),
        }
        _CACHE["w_dev"] = {k: rt["device_put"](v, rt["shc"]) for k, v in w_np.items()}
        _CACHE["wkey"] = (Wq_f.copy(), Wk_f.copy(), Wv_f.copy(), Wo_f.copy())

    # ---- host LayerNorm (exact f32) + fp8 quantize + per-core slicing ----
    B = x.shape[0]
    N = x.shape[2] * x.shape[3]
    xf = x.reshape(B, C, N)
    xn = _layernorm_bcx(xf, ln_x_g, ln_x_b)          # [B, C, N] f32, exact
    xn8 = xn.astype(F8NP)
    xn_cat = np.empty((8 * C, NQ), F8NP)
    for core in range(8):
        b, hf = core // 2, core % 2
        xn_cat[C * core:C * (core + 1)] = xn8[b, :, hf * NQ:(hf + 1) * NQ]

    ynt = np.ascontiguousarray(y.swapaxes(1, 2))     # [B, C, M]
    yn = _layernorm_bcx(ynt, ln_y_g, ln_y_b)
    yn8 = yn.astype(F8NP)
    yn_cat = np.empty((8 * C, M), F8NP)
    for core in range(8):
        yn_cat[C * core:C * (core + 1)] = yn8[core // 2]

    # ---- donated output buffer: recycle previous call's device buffer ----
    zeros = _CACHE.pop("recycle_out", None)
    if zeros is None:
        zeros = np.zeros((8 * C, NQ), F8NP)

    args = {"xn": xn_cat, "yn": yn_cat, **_CACHE["w_dev"], "out": zeros}
    out_arrs = rt["sharded"](*[args[n] for n in rt["in_names"]], args["out"])

    oa = np.asarray(out_arrs[0])                     # [8*C, NQ] fp8, blocks host
    _CACHE["recycle_out"] = out_arrs[0]

    # ---- residual + biases on host, f32 exact ----
    oa4 = oa.reshape(B, 2, C, NQ)
    final = np.empty((B, C, N), np.float32)
    fv = final.reshape(B, C, 2, NQ)
    fv[:, :, 0, :] = oa4[:, 0]                       # fp8 -> f32 fused in assign
    fv[:, :, 1, :] = oa4[:, 1]
    final += xn
    if (bv_f != 0.0).any() or (bo_f != 0.0).any():
        final += (bv_f @ Wo_f + bo_f)[None, :, None]
    return final.swapaxes(1, 2)                      # [B, N, C] view


# revision 6
# speedup vs baseline: 5.6149x; 1.3678x over previous
"""CABlock cross-attention kernel for 8 TRN2 NeuronCores.

Sharding: 8 cores = 4 batches x 2 query-halves; each core computes an
independent output slice, no collectives.

The axon tunnel (~50MB/s up, ~33MB/s down) dominates wall time, so the
host/device split minimizes bytes on the wire:
  - LayerNorm (incl. gains/biases) runs on host in f32; normalized
    activations ship as fp8 e4m3 in c-layout [C, N] (natural layout of x,
    so no transposes anywhere).
  - The device computes attention only and returns o@Wo in fp8 c-layout.
  - The residual add (exact f32 xn) happens on host; output biases
    (bv@Wo + bo) are folded in on host exactly.
  - Weights (bf16) are uploaded once and cached on device; the jitted
    executable is cached; the donated output buffer is recycled from the
    previous call, so steady-state traffic is 6MB up + 4MB down.
"""

import sys

import numpy as np

try:
    import concourse.bass as bass  # noqa: F401
except ImportError:
    sys.path.insert(0, "/opt/trn_rl_repo")
    import concourse.bass as bass

import ml_dtypes
import concourse.mybir as mybir
import concourse.tile as tile

F32 = mybir.dt.float32
BF16 = mybir.dt.bfloat16
F8 = mybir.dt.float8e4
BF = ml_dtypes.bfloat16
F8NP = ml_dtypes.float8_e4m3

# per-core problem dims
NQ = 2048   # query rows per core (16 tiles of 128)
M = 1024    # context rows (8 tiles of 128)
C = 256     # model dim (2 chunks of 128)
INNER = 512  # heads*dim_head (4 chunks of 128)
H = 8       # heads
DH = 64     # dim_head
NQT = NQ // 128   # 16
MT = M // 128     # 8
CC = C // 128     # 2
IC = INNER // 128  # 4
EPS = 1e-5

_CACHE = {}


def _split_multiwaits(nc):
    """walrus allows only one sem-wait per ISA instruction; move extra waits
    onto same-engine NoOps inserted immediately before the instruction."""
    cnt = 0
    for f in nc.m.functions:
        for b in f.blocks:
            out = []
            for inst in b.instructions:
                si = inst.sync_info
                if si is not None and si.on_wait and len(si.on_wait) > 1:
                    waits = list(si.on_wait)
                    for w in waits[:-1]:
                        cnt += 1
                        nop = mybir.InstNoOp(
                            name=f"WSPLIT-{cnt}",
                            ins=[], outs=[],
                            engine=inst.engine,
                            sync_info=mybir.SyncInfo(on_wait=[w], on_update=[]),
                            bass_nofuse=True,
                        )
                        out.append(nop)
                    inst.sync_info = mybir.SyncInfo(
                        on_wait=[waits[-1]], on_update=list(si.on_update)
                    )
                out.append(inst)
            b.instructions = out
    return nc


def _build_nc():
    from concourse.masks import make_identity

    nc = bass.Bass()
    x_ext = nc.declare_dram_parameter("xn", [C, NQ], F8, isOutput=False)
    y_ext = nc.declare_dram_parameter("yn", [C, M], F8, isOutput=False)
    wq_ext = nc.declare_dram_parameter("wq", [C, INNER], BF16, isOutput=False)
    wk_ext = nc.declare_dram_parameter("wk", [C, INNER], BF16, isOutput=False)
    wv_ext = nc.declare_dram_parameter("wv", [C, INNER], BF16, isOutput=False)
    wo_ext = nc.declare_dram_parameter("wo", [INNER, C], BF16, isOutput=False)
    out_ext = nc.declare_dram_parameter("out", [C, NQ], F8, isOutput=True)

    with tile.TileContext(nc) as tc:
        with (
            tc.tile_pool(name="singles", bufs=1) as singles,
            tc.tile_pool(name="big", bufs=1) as big,
            tc.tile_pool(name="probs", bufs=4) as probs_pool,
            tc.tile_pool(name="stats", bufs=4) as stats,
            tc.tile_pool(name="ps_big", bufs=2, space="PSUM") as ps_big,
            tc.tile_pool(name="ps_small", bufs=4, space="PSUM") as ps_small,
        ):
            ident_bf = singles.tile([128, 128], BF16)
            make_identity(nc, ident_bf)

            # weights (device-HBM resident across calls; cheap DMA here)
            wq_sb = singles.tile([128, CC, INNER], BF16)
            nc.gpsimd.dma_start(wq_sb, wq_ext.rearrange("(kc p) i -> p kc i", p=128))
            wk_sb = singles.tile([128, CC, INNER], BF16)
            nc.gpsimd.dma_start(wk_sb, wk_ext.rearrange("(kc p) i -> p kc i", p=128))
            wv_sb = singles.tile([128, CC, INNER], BF16)
            nc.gpsimd.dma_start(wv_sb, wv_ext.rearrange("(kc p) i -> p kc i", p=128))
            wo_sb = singles.tile([128, IC, C], BF16)
            nc.gpsimd.dma_start(wo_sb, wo_ext.rearrange("(ic p) c -> p ic c", p=128))

            # activations: host-LayerNormed, fp8, c-layout -- no transposes
            xnT = big.tile([128, CC, NQ], F8)
            nc.gpsimd.dma_start(xnT, x_ext.rearrange("(kc p) n -> p kc n", p=128))
            ynT = big.tile([128, CC, M], F8)
            nc.gpsimd.dma_start(ynT, y_ext.rearrange("(kc p) n -> p kc n", p=128))

            # ---- projections (PE reads fp8 activations directly) ----
            # qT[inner, nq]
            qt = big.tile([128, IC, NQ], BF16)
            for ic in range(IC):
                for nqc in range(NQ // 512):
                    pq = ps_small.tile([128, 512], F32, tag="ps_sm")
                    for kc in range(CC):
                        nc.tensor.matmul(
                            pq, lhsT=wq_sb[:, kc, ic * 128:(ic + 1) * 128],
                            rhs=xnT[:, kc, nqc * 512:(nqc + 1) * 512],
                            start=(kc == 0), stop=(kc == CC - 1),
                        )
                    nc.vector.tensor_copy(out=qt[:, ic, nqc * 512:(nqc + 1) * 512], in_=pq)
            # kT[inner, m]
            kt = big.tile([128, IC, M], BF16)
            for ic in range(IC):
                for mc in range(M // 512):
                    pk = ps_small.tile([128, 512], F32, tag="ps_sm")
                    for kc in range(CC):
                        nc.tensor.matmul(
                            pk, lhsT=wk_sb[:, kc, ic * 128:(ic + 1) * 128],
                            rhs=ynT[:, kc, mc * 512:(mc + 1) * 512],
                            start=(kc == 0), stop=(kc == CC - 1),
                        )
                    nc.vector.tensor_copy(out=kt[:, ic, mc * 512:(mc + 1) * 512], in_=pk)
            # v[m, h, 65]  (col 64 = ones for row-sums)
            v_sb = big.tile([128, MT, H, DH + 1], BF16)
            nc.vector.memset(v_sb[:, :, :, DH:DH + 1], 1.0)
            for mt in range(MT):
                pv = ps_small.tile([128, 512], F32, tag="ps_sm")
                for kc in range(CC):
                    nc.tensor.matmul(
                        pv, lhsT=ynT[:, kc, mt * 128:(mt + 1) * 128],
                        rhs=wv_sb[:, kc, :],
                        start=(kc == 0), stop=(kc == CC - 1),
                    )
                nc.vector.tensor_copy(
                    out=v_sb[:, mt, :, 0:DH],
                    in_=pv.rearrange("p (h e) -> p h e", h=H),
                )

            # ---- attention, head pairs ----
            o_sb = big.tile([128, NQT, IC, 128], BF16)  # o[nq, inner]
            for hp in range(H // 2):
                for nqh in range(2):  # nq halves pipeline independently
                    pT = []
                    for hh in range(2):
                        pT.append(probs_pool.tile([128, MT, NQ // 2], BF16,
                                                  tag="probsT",
                                                  name=f"probsT_{hp}_{nqh}_{hh}"))
                    # scoresT + exp:  ET[nk, nq] = kT_h[:,nk_tile].T @ qT_h
                    for mt in range(MT):
                        pe = []
                        for hh in range(2):
                            p_e = ps_big.tile([128, 1024], F32, tag="escore")
                            lhsT = kt[hh * 64:(hh + 1) * 64, hp, mt * 128:(mt + 1) * 128]
                            for n2 in range(2):
                                nc.tensor.matmul(
                                    p_e[:, n2 * 512:(n2 + 1) * 512],
                                    lhsT=lhsT,
                                    rhs=qt[hh * 64:(hh + 1) * 64, hp,
                                           nqh * 1024 + n2 * 512:nqh * 1024 + (n2 + 1) * 512],
                                    start=True, stop=True,
                                )
                            pe.append(p_e)
                        for hh in range(2):
                            nc.scalar.activation(
                                out=pT[hh][:, mt, :],
                                in_=pe[hh],
                                func=mybir.ActivationFunctionType.Exp,
                            )
                    # attn@v: o[nq_tile, 65] = probsT[:,nq_tile].T @ v_aug
                    for lq in range(NQT // 2):
                        nqt = nqh * (NQT // 2) + lq
                        for hh in range(2):
                            h = hp * 2 + hh
                            po = ps_small.tile([128, 512], F32, tag="ps_sm")
                            for mt in range(MT):
                                nc.tensor.matmul(
                                    po[:, :DH + 1],
                                    lhsT=pT[hh][:, mt, lq * 128:(lq + 1) * 128],
                                    rhs=v_sb[:, mt, h, :],
                                    start=(mt == 0), stop=(mt == MT - 1),
                                )
                            rs = stats.tile([128, 1], F32, tag="rs")
                            nc.vector.reciprocal(out=rs, in_=po[:, DH:DH + 1])
                            nc.vector.tensor_scalar_mul(
                                out=o_sb[:, nqt, h // 2, (h % 2) * DH:(h % 2) * DH + DH],
                                in0=po[:, 0:DH], scalar1=rs,
                            )

            # ---- transpose o -> oT[inner, nq] ----
            oT = big.tile([128, IC, NQ], BF16)
            for ic in range(IC):
                for nqt in range(NQT):
                    pt = ps_small.tile([128, 512], BF16, tag="ps_sm")
                    nc.tensor.transpose(pt[:, :128], o_sb[:, nqt, ic, :], ident_bf)
                    nc.vector.tensor_copy(out=oT[:, ic, nqt * 128:(nqt + 1) * 128], in_=pt[:, :128])

            # ---- out-proj, c-layout (lhsT/rhs swapped => no output transpose) ----
            out_sb = big.tile([128, CC, NQ], F8)
            for cc in range(CC):
                for nqc in range(NQ // 512):
                    pf = ps_small.tile([128, 512], F32, tag="ps_sm")
                    for ic in range(IC):
                        nc.tensor.matmul(
                            pf,
                            lhsT=wo_sb[:, ic, cc * 128:(cc + 1) * 128],
                            rhs=oT[:, ic, nqc * 512:(nqc + 1) * 512],
                            start=(ic == 0), stop=(ic == IC - 1),
                        )
                    nc.vector.tensor_copy(out=out_sb[:, cc, nqc * 512:(nqc + 1) * 512], in_=pf)
            nc.gpsimd.dma_start(out_ext.rearrange("(cc p) n -> p cc n", p=128), out_sb)
    return _split_multiwaits(nc)


def _get_runtime():
    """Build (once) the Bass module, a cached jitted SPMD executor, and
    jitted CPU helpers for LayerNorm/quantize/dequantize."""
    if "sharded" in _CACHE:
        return _CACHE
    import jax
    import jax.numpy as jnp
    from jax.sharding import Mesh, PartitionSpec, NamedSharding
    try:
        from jax.experimental.shard_map import shard_map
    except ImportError:
        from jax import shard_map
    from concourse.bass2jax import (
        _bass_exec_p, install_neuronx_cc_hook, partition_id_tensor,
    )

    nc = _build_nc()
    install_neuronx_cc_hook()

    partition_name = nc.partition_id_tensor.name if nc.partition_id_tensor else None
    in_names, out_names, out_avals = [], [], []
    for alloc in nc.m.functions[0].allocations:
        if not isinstance(alloc, mybir.MemoryLocationSet):
            continue
        name = alloc.memorylocations[0].name
        if alloc.kind == "ExternalInput":
            if name != partition_name:
                in_names.append(name)
        elif alloc.kind == "ExternalOutput":
            out_names.append(name)
            out_avals.append(jax.core.ShapedArray(
                tuple(alloc.tensor_shape), mybir.dt.np(alloc.dtype)))
    n_params = len(in_names)
    n_outs = len(out_avals)
    in_names_full = in_names + out_names + ([partition_name] if partition_name else [])

    def _body(*args):
        operands = list(args)
        if partition_name is not None:
            operands.append(partition_id_tensor())
        outs = _bass_exec_p.bind(
            *operands,
            out_avals=tuple(out_avals),
            in_names=tuple(in_names_full),
            out_names=tuple(out_names),
            lowering_input_output_aliases=(),
            sim_require_finite=True,
            sim_require_nnan=True,
            nc=nc,
        )
        return tuple(outs)

    devices = jax.devices()[:8]
    mesh = Mesh(np.asarray(devices), ("core",))
    shc = NamedSharding(mesh, PartitionSpec("core"))
    donate = tuple(range(n_params, n_params + n_outs))
    sharded = jax.jit(
        shard_map(_body, mesh=mesh,
                  in_specs=(PartitionSpec("core"),) * (n_params + n_outs),
                  out_specs=(PartitionSpec("core"),) * n_outs,
                  check_rep=False),
        donate_argnums=donate, keep_unused=True,
    )
    cpu = jax.devices("cpu")[0]
    F8J = jnp.float8_e4m3

    def _prep_x(xr, g, b):
        # [B, C, N] -> exact f32 LN (over C) and fp8 per-core concat
        mu = xr.mean(axis=1, keepdims=True)
        var = (xr * xr).mean(axis=1, keepdims=True) - mu * mu
        xn = (xr - mu) * jax.lax.rsqrt(var + EPS)
        xn = xn * g[:, None] + b[:, None]
        cat = (xn.astype(F8J).reshape(4, C, 2, NQ)
               .transpose(0, 2, 1, 3).reshape(8 * C, NQ))
        return xn, cat

    def _prep_y(yr, g, b):
        # [B, M, C] -> fp8 per-core concat of LN(y)^T (duplicated per pair)
        mu = yr.mean(axis=-1, keepdims=True)
        var = (yr * yr).mean(axis=-1, keepdims=True) - mu * mu
        yn = (yr - mu) * jax.lax.rsqrt(var + EPS) * g + b
        ynt8 = yn.transpose(0, 2, 1).astype(F8J)
        return jnp.repeat(ynt8, 2, axis=0).reshape(8 * C, M)

    def _post(oa, xn, cst):
        # fp8 attention output (c-layout blocks) + exact residual + biases
        o = (oa.reshape(4, 2, C, NQ).astype(jnp.float32)
             .transpose(0, 2, 1, 3).reshape(4, C, 2 * NQ))
        return o + xn + cst[None, :, None]

    def _on_cpu(fn):
        jfn = jax.jit(fn)

        def run(*a):
            with jax.default_device(cpu):
                return jfn(*a)
        return run

    _CACHE.update(nc=nc, sharded=sharded, in_names=in_names, shc=shc,
                  jax=jax, device_put=jax.device_put,
                  prep_x=_on_cpu(_prep_x), prep_y=_on_cpu(_prep_y),
                  post=_on_cpu(_post))
    return _CACHE


def kernel(x, y, ln_x_g, ln_x_b, ln_y_g, ln_y_b, Wq, Wk, Wv, bv, Wo, bo, **kw):
    x = np.asarray(x, np.float32)
    y = np.asarray(y, np.float32)
    ln_x_g = np.asarray(ln_x_g, np.float32)
    ln_x_b = np.asarray(ln_x_b, np.float32)
    ln_y_g = np.asarray(ln_y_g, np.float32)
    ln_y_b = np.asarray(ln_y_b, np.float32)
    Wq_f = np.asarray(Wq, np.float32)
    Wk_f = np.asarray(Wk, np.float32)
    Wv_f = np.asarray(Wv, np.float32)
    Wo_f = np.asarray(Wo, np.float32)
    bv_f = np.asarray(bv, np.float32)
    bo_f = np.asarray(bo, np.float32)

    rt = _get_runtime()

    # ---- device-resident weights (re-uploaded only when they change) ----
    wkey = _CACHE.get("wkey")
    if wkey is None or not all(
        np.array_equal(a, b) for a, b in zip(wkey, (Wq_f, Wk_f, Wv_f, Wo_f))
    ):
        s = DH ** -0.5
        w_np = {
            "wq": np.concatenate([(Wq_f * s).astype(BF)] * 8, axis=0),
            "wk": np.concatenate([Wk_f.astype(BF)] * 8, axis=0),
            "wv": np.concatenate([Wv_f.astype(BF)] * 8, axis=0),
            "wo": np.concatenate([Wo_f.astype(BF)] * 8, axis=0),
        }
        _CACHE["w_dev"] = {k: rt["device_put"](v, rt["shc"]) for k, v in w_np.items()}
        _CACHE["wkey"] = (Wq_f.copy(), Wk_f.copy(), Wv_f.copy(), Wo_f.copy())

    # ---- host LayerNorm (exact f32) + fp8 quantize on the CPU backend;
    # upload of xn starts async while the y-side prep still runs ----
    B = x.shape[0]
    N = x.shape[2] * x.shape[3]
    xf = x.reshape(B, C, N)
    xn, xn_cat = rt["prep_x"](xf, ln_x_g, ln_x_b)
    xd = rt["device_put"](np.asarray(xn_cat), rt["shc"])   # async upload
    yn_cat = rt["prep_y"](y, ln_y_g, ln_y_b)
    yd = rt["device_put"](np.asarray(yn_cat), rt["shc"])

    # donated output buffer: recycle previous call's device buffer
    zeros = _CACHE.pop("recycle_out", None)
    if zeros is None:
        zeros = np.zeros((8 * C, NQ), F8NP)

    args = {"xn": xd, "yn": yd, **_CACHE["w_dev"]}
    out_arrs = rt["sharded"](*[args[n] for n in rt["in_names"]], zeros)

    oa = np.asarray(out_arrs[0])                     # [8*C, NQ] fp8, blocks host
    _CACHE["recycle_out"] = out_arrs[0]

    # ---- residual + biases on host, f32 exact ----
    cst = bv_f @ Wo_f + bo_f
    final = np.asarray(rt["post"](oa, xn, cst))      # [B, C, N] f32
    return final.swapaxes(1, 2)                      # [B, N, C] view


# revision 7
# speedup vs baseline: 6.4369x; 1.1464x over previous
"""CABlock cross-attention kernel for 8 TRN2 NeuronCores.

Sharding: 8 cores = 4 batches x 2 query-halves; each core computes an
independent output slice, no collectives.

The axon tunnel (~30-60MB/s each way, full-duplex) dominates wall time,
so the host/device split minimizes and overlaps bytes on the wire:
  - LayerNorm (incl. gains/biases) runs on host (jitted on the CPU
    backend); normalized activations ship as fp8 e4m3 in c-layout [C, N]
    (the natural layout of x, so no transposes anywhere).
  - Device work is split into a KV program (yn -> k,v, kept
    device-resident) and a Q program (xn chunk -> fp8 attention output
    chunk) dispatched K times; chunk uploads overlap earlier chunks'
    downloads on the duplex tunnel.
  - The residual add (exact f32 xn) and output biases (bv@Wo + bo)
    happen on host.
  - Weights (bf16) upload once and stay device-resident; jitted
    executables are cached; output operands are persistent device
    dummies (both programs write every output element, so no donated
    zero buffers ever cross the wire).
"""

import sys

import numpy as np

try:
    import concourse.bass as bass  # noqa: F401
except ImportError:
    sys.path.insert(0, "/opt/trn_rl_repo")
    import concourse.bass as bass

import ml_dtypes
import concourse.mybir as mybir
import concourse.tile as tile

F32 = mybir.dt.float32
BF16 = mybir.dt.bfloat16
F8 = mybir.dt.float8e4
BF = ml_dtypes.bfloat16
F8NP = ml_dtypes.float8_e4m3

# per-core problem dims
NQ = 2048   # query rows per core (16 tiles of 128)
M = 1024    # context rows (8 tiles of 128)
C = 256     # model dim (2 chunks of 128)
INNER = 512  # heads*dim_head (4 chunks of 128)
H = 8       # heads
DH = 64     # dim_head
NQT = NQ // 128   # 16
MT = M // 128     # 8
CC = C // 128     # 2
IC = INNER // 128  # 4
EPS = 1e-5

K_CHUNKS = 4
QCH = NQ // K_CHUNKS       # query cols per chunk per core
QCT = QCH // 128           # q tiles per chunk

_CACHE = {}


def _split_multiwaits(nc):
    """walrus allows only one sem-wait per ISA instruction; move extra waits
    onto same-engine NoOps inserted immediately before the instruction."""
    cnt = 0
    for f in nc.m.functions:
        for b in f.blocks:
            out = []
            for inst in b.instructions:
                si = inst.sync_info
                if si is not None and si.on_wait and len(si.on_wait) > 1:
                    waits = list(si.on_wait)
                    for w in waits[:-1]:
                        cnt += 1
                        nop = mybir.InstNoOp(
                            name=f"WSPLIT-{cnt}",
                            ins=[], outs=[],
                            engine=inst.engine,
                            sync_info=mybir.SyncInfo(on_wait=[w], on_update=[]),
                            bass_nofuse=True,
                        )
                        out.append(nop)
                    inst.sync_info = mybir.SyncInfo(
                        on_wait=[waits[-1]], on_update=list(si.on_update)
                    )
                out.append(inst)
            b.instructions = out
    return nc


def _build_kv_nc():
    """yn (fp8, c-layout) -> kT [INNER, M] bf16, v_aug [M, H*(DH+1)] bf16."""
    nc = bass.Bass()
    y_ext = nc.declare_dram_parameter("yn", [C, M], F8, isOutput=False)
    wk_ext = nc.declare_dram_parameter("wk", [C, INNER], BF16, isOutput=False)
    wv_ext = nc.declare_dram_parameter("wv", [C, INNER], BF16, isOutput=False)
    kt_ext = nc.declare_dram_parameter("kt", [INNER, M], BF16, isOutput=True)
    v_ext = nc.declare_dram_parameter("v", [M, H * (DH + 1)], BF16, isOutput=True)

    with tile.TileContext(nc) as tc:
        with (
            tc.tile_pool(name="sb", bufs=1) as sb,
            tc.tile_pool(name="ps", bufs=4, space="PSUM") as ps,
        ):
            wk_sb = sb.tile([128, CC, INNER], BF16)
            nc.gpsimd.dma_start(wk_sb, wk_ext.rearrange("(kc p) i -> p kc i", p=128))
            wv_sb = sb.tile([128, CC, INNER], BF16)
            nc.gpsimd.dma_start(wv_sb, wv_ext.rearrange("(kc p) i -> p kc i", p=128))
            ynT = sb.tile([128, CC, M], F8)
            nc.gpsimd.dma_start(ynT, y_ext.rearrange("(kc p) n -> p kc n", p=128))

            kt = sb.tile([128, IC, M], BF16)
            for ic in range(IC):
                for mc in range(M // 512):
                    pk = ps.tile([128, 512], F32, tag="ps")
                    for kc in range(CC):
                        nc.tensor.matmul(
                            pk, lhsT=wk_sb[:, kc, ic * 128:(ic + 1) * 128],
                            rhs=ynT[:, kc, mc * 512:(mc + 1) * 512],
                            start=(kc == 0), stop=(kc == CC - 1),
                        )
                    nc.vector.tensor_copy(out=kt[:, ic, mc * 512:(mc + 1) * 512], in_=pk)
            nc.gpsimd.dma_start(kt_ext.rearrange("(ic p) m -> p ic m", p=128), kt)

            v_sb = sb.tile([128, MT, H, DH + 1], BF16)
            nc.vector.memset(v_sb[:, :, :, DH:DH + 1], 1.0)
            for mt in range(MT):
                pv = ps.tile([128, 512], F32, tag="ps")
                for kc in range(CC):
                    nc.tensor.matmul(
                        pv, lhsT=ynT[:, kc, mt * 128:(mt + 1) * 128],
                        rhs=wv_sb[:, kc, :],
                        start=(kc == 0), stop=(kc == CC - 1),
                    )
                nc.vector.tensor_copy(
                    out=v_sb[:, mt, :, 0:DH],
                    in_=pv.rearrange("p (h e) -> p h e", h=H),
                )
            nc.gpsimd.dma_start(
                v_ext.rearrange("(mt p) (h e) -> p mt h e", p=128, h=H), v_sb
            )
    return _split_multiwaits(nc)


def _build_q_nc():
    """xn chunk (fp8, c-layout) + device-resident kT/v -> out chunk fp8."""
    nc = bass.Bass()
    x_ext = nc.declare_dram_parameter("xn", [C, QCH], F8, isOutput=False)
    kt_ext = nc.declare_dram_parameter("kt", [INNER, M], BF16, isOutput=False)
    v_ext = nc.declare_dram_parameter("v", [M, H * (DH + 1)], BF16, isOutput=False)
    wq_ext = nc.declare_dram_parameter("wq", [C, INNER], BF16, isOutput=False)
    wo_ext = nc.declare_dram_parameter("wo", [INNER, C], BF16, isOutput=False)
    out_ext = nc.declare_dram_parameter("out", [C, QCH], F8, isOutput=True)

    from concourse.masks import make_identity

    with tile.TileContext(nc) as tc:
        with (
            tc.tile_pool(name="sb", bufs=1) as sb,
            tc.tile_pool(name="probs", bufs=4) as probs_pool,
            tc.tile_pool(name="stats", bufs=4) as stats,
            tc.tile_pool(name="ps_big", bufs=2, space="PSUM") as ps_big,
            tc.tile_pool(name="ps_small", bufs=4, space="PSUM") as ps_small,
        ):
            ident_bf = sb.tile([128, 128], BF16)
            make_identity(nc, ident_bf)

            wq_sb = sb.tile([128, CC, INNER], BF16)
            nc.gpsimd.dma_start(wq_sb, wq_ext.rearrange("(kc p) i -> p kc i", p=128))
            wo_sb = sb.tile([128, IC, C], BF16)
            nc.gpsimd.dma_start(wo_sb, wo_ext.rearrange("(ic p) c -> p ic c", p=128))
            kt = sb.tile([128, IC, M], BF16)
            nc.gpsimd.dma_start(kt, kt_ext.rearrange("(ic p) m -> p ic m", p=128))
            v_sb = sb.tile([128, MT, H, DH + 1], BF16)
            nc.gpsimd.dma_start(
                v_sb, v_ext.rearrange("(mt p) (h e) -> p mt h e", p=128, h=H)
            )
            xnT = sb.tile([128, CC, QCH], F8)
            nc.gpsimd.dma_start(xnT, x_ext.rearrange("(kc p) n -> p kc n", p=128))

            # qT[inner, qch]
            qt = sb.tile([128, IC, QCH], BF16)
            for ic in range(IC):
                for nqc in range(QCH // 512):
                    pq = ps_small.tile([128, 512], F32, tag="ps_sm")
                    for kc in range(CC):
                        nc.tensor.matmul(
                            pq, lhsT=wq_sb[:, kc, ic * 128:(ic + 1) * 128],
                            rhs=xnT[:, kc, nqc * 512:(nqc + 1) * 512],
                            start=(kc == 0), stop=(kc == CC - 1),
                        )
                    nc.vector.tensor_copy(out=qt[:, ic, nqc * 512:(nqc + 1) * 512], in_=pq)

            # attention, head pairs
            o_sb = sb.tile([128, QCT, IC, 128], BF16)  # o[nq, inner]
            for hp in range(H // 2):
                pT = []
                for hh in range(2):
                    pT.append(probs_pool.tile([128, MT, QCH], BF16,
                                              tag="probsT", name=f"probsT_{hp}_{hh}"))
                for mt in range(MT):
                    pe = []
                    for hh in range(2):
                        p_e = ps_big.tile([128, QCH], F32, tag="escore")
                        lhsT = kt[hh * 64:(hh + 1) * 64, hp, mt * 128:(mt + 1) * 128]
                        for n2 in range(QCH // 512):
                            nc.tensor.matmul(
                                p_e[:, n2 * 512:(n2 + 1) * 512],
                                lhsT=lhsT,
                                rhs=qt[hh * 64:(hh + 1) * 64, hp,
                                       n2 * 512:(n2 + 1) * 512],
                                start=True, stop=True,
                            )
                        pe.append(p_e)
                    for hh in range(2):
                        nc.scalar.activation(
                            out=pT[hh][:, mt, :],
                            in_=pe[hh],
                            func=mybir.ActivationFunctionType.Exp,
                        )
                for lq in range(QCT):
                    for hh in range(2):
                        h = hp * 2 + hh
                        po = ps_small.tile([128, 512], F32, tag="ps_sm")
                        for mt in range(MT):
                            nc.tensor.matmul(
                                po[:, :DH + 1],
                                lhsT=pT[hh][:, mt, lq * 128:(lq + 1) * 128],
                                rhs=v_sb[:, mt, h, :],
                                start=(mt == 0), stop=(mt == MT - 1),
                            )
                        rs = stats.tile([128, 1], F32, tag="rs")
                        nc.vector.reciprocal(out=rs, in_=po[:, DH:DH + 1])
                        nc.vector.tensor_scalar_mul(
                            out=o_sb[:, lq, h // 2, (h % 2) * DH:(h % 2) * DH + DH],
                            in0=po[:, 0:DH], scalar1=rs,
                        )

            # transpose o -> oT[inner, nq]
            oT = sb.tile([128, IC, QCH], BF16)
            for ic in range(IC):
                for nqt in range(QCT):
                    pt = ps_small.tile([128, 512], BF16, tag="ps_sm")
                    nc.tensor.transpose(pt[:, :128], o_sb[:, nqt, ic, :], ident_bf)
                    nc.vector.tensor_copy(out=oT[:, ic, nqt * 128:(nqt + 1) * 128],
                                          in_=pt[:, :128])

            # out-proj, c-layout (lhsT/rhs swapped => no output transpose)
            out_sb = sb.tile([128, CC, QCH], F8)
            for cc in range(CC):
                for nqc in range(QCH // 512):
                    pf = ps_small.tile([128, 512], F32, tag="ps_sm")
                    for ic in range(IC):
                        nc.tensor.matmul(
                            pf,
                            lhsT=wo_sb[:, ic, cc * 128:(cc + 1) * 128],
                            rhs=oT[:, ic, nqc * 512:(nqc + 1) * 512],
                            start=(ic == 0), stop=(ic == IC - 1),
                        )
                    nc.vector.tensor_copy(out=out_sb[:, cc, nqc * 512:(nqc + 1) * 512],
                                          in_=pf)
            nc.gpsimd.dma_start(out_ext.rearrange("(cc p) n -> p cc n", p=128), out_sb)
    return _split_multiwaits(nc)


def _make_sharded(nc, jax, shard_map, mesh, PartitionSpec):
    """Jitted SPMD executor for a Bass module; output operands are plain
    (non-donated) parameters, so persistent dummies can back them."""
    from concourse.bass2jax import _bass_exec_p, partition_id_tensor

    partition_name = nc.partition_id_tensor.name if nc.partition_id_tensor else None
    in_names, out_names, out_avals = [], [], []
    for alloc in nc.m.functions[0].allocations:
        if not isinstance(alloc, mybir.MemoryLocationSet):
            continue
        name = alloc.memorylocations[0].name
        if alloc.kind == "ExternalInput":
            if name != partition_name:
                in_names.append(name)
        elif alloc.kind == "ExternalOutput":
            out_names.append(name)
            out_avals.append(jax.core.ShapedArray(
                tuple(alloc.tensor_shape), mybir.dt.np(alloc.dtype)))
    n_params = len(in_names)
    n_outs = len(out_avals)
    in_names_full = in_names + out_names + ([partition_name] if partition_name else [])

    def _body(*args):
        operands = list(args)
        if partition_name is not None:
            operands.append(partition_id_tensor())
        outs = _bass_exec_p.bind(
            *operands,
            out_avals=tuple(out_avals),
            in_names=tuple(in_names_full),
            out_names=tuple(out_names),
            lowering_input_output_aliases=(),
            sim_require_finite=True,
            sim_require_nnan=True,
            nc=nc,
        )
        return tuple(outs)

    sharded = jax.jit(
        shard_map(_body, mesh=mesh,
                  in_specs=(PartitionSpec("core"),) * (n_params + n_outs),
                  out_specs=(PartitionSpec("core"),) * n_outs,
                  check_rep=False),
        keep_unused=True,
    )
    return sharded, in_names, [(tuple(a.shape), a.dtype) for a in out_avals]


def _get_runtime():
    if "q_sharded" in _CACHE:
        return _CACHE
    import jax
    import jax.numpy as jnp
    from jax.sharding import Mesh, PartitionSpec, NamedSharding
    try:
        from jax.experimental.shard_map import shard_map
    except ImportError:
        from jax import shard_map
    from concourse.bass2jax import install_neuronx_cc_hook

    install_neuronx_cc_hook()
    devices = jax.devices()[:8]
    mesh = Mesh(np.asarray(devices), ("core",))
    shc = NamedSharding(mesh, PartitionSpec("core"))

    kv_sharded, kv_in, kv_outspec = _make_sharded(
        _build_kv_nc(), jax, shard_map, mesh, PartitionSpec)
    q_sharded, q_in, q_outspec = _make_sharded(
        _build_q_nc(), jax, shard_map, mesh, PartitionSpec)

    # persistent device dummies backing the output operands
    def dev_zeros(spec):
        return jax.jit(
            lambda: tuple(jnp.zeros((8 * s[0], *s[1:]), d) for s, d in spec),
            out_shardings=(shc,) * len(spec))()
    kv_dummy = dev_zeros(kv_outspec)
    q_dummy = dev_zeros(q_outspec)
    jax.block_until_ready((kv_dummy, q_dummy))

    cpu = jax.devices("cpu")[0]
    F8J = jnp.float8_e4m3

    def _prep_x(xr, g, b):
        # [B, C, N] -> exact f32 LN (over C) and per-chunk fp8 concats
        mu = xr.mean(axis=1, keepdims=True)
        var = (xr * xr).mean(axis=1, keepdims=True) - mu * mu
        xn = (xr - mu) * jax.lax.rsqrt(var + EPS)
        xn = xn * g[:, None] + b[:, None]
        # (b, c, hf, k, i) -> (k, b*hf*c, i)
        cat = (xn.astype(F8J).reshape(4, C, 2, K_CHUNKS, QCH)
               .transpose(3, 0, 2, 1, 4).reshape(K_CHUNKS, 8 * C, QCH))
        return xn, cat

    def _prep_y(yr, g, b):
        mu = yr.mean(axis=-1, keepdims=True)
        var = (yr * yr).mean(axis=-1, keepdims=True) - mu * mu
        yn = (yr - mu) * jax.lax.rsqrt(var + EPS) * g + b
        ynt8 = yn.transpose(0, 2, 1).astype(F8J)
        return jnp.repeat(ynt8, 2, axis=0).reshape(8 * C, M)

    def _post(oas, xn, cst):
        # oas [K, 8C, QCH] fp8 -> [B, C, N] f32 + residual + biases
        o = (oas.reshape(K_CHUNKS, 4, 2, C, QCH).astype(jnp.float32)
             .transpose(1, 3, 2, 0, 4).reshape(4, C, 2 * NQ))
        return o + xn + cst[None, :, None]

    def _on_cpu(fn):
        jfn = jax.jit(fn)

        def run(*a):
            with jax.default_device(cpu):
                return jfn(*a)
        return run

    _CACHE.update(
        kv_sharded=kv_sharded, kv_in=kv_in, kv_dummy=kv_dummy,
        q_sharded=q_sharded, q_in=q_in, q_dummy=q_dummy,
        shc=shc, jax=jax, device_put=jax.device_put,
        prep_x=_on_cpu(_prep_x), prep_y=_on_cpu(_prep_y), post=_on_cpu(_post),
    )
    return _CACHE


def kernel(x, y, ln_x_g, ln_x_b, ln_y_g, ln_y_b, Wq, Wk, Wv, bv, Wo, bo, **kw):
    x = np.asarray(x, np.float32)
    y = np.asarray(y, np.float32)
    ln_x_g = np.asarray(ln_x_g, np.float32)
    ln_x_b = np.asarray(ln_x_b, np.float32)
    ln_y_g = np.asarray(ln_y_g, np.float32)
    ln_y_b = np.asarray(ln_y_b, np.float32)
    Wq_f = np.asarray(Wq, np.float32)
    Wk_f = np.asarray(Wk, np.float32)
    Wv_f = np.asarray(Wv, np.float32)
    Wo_f = np.asarray(Wo, np.float32)
    bv_f = np.asarray(bv, np.float32)
    bo_f = np.asarray(bo, np.float32)

    rt = _get_runtime()

    # device-resident weights (re-uploaded only when they change)
    wkey = _CACHE.get("wkey")
    if wkey is None or not all(
        np.array_equal(wa, wb) for wa, wb in zip(wkey, (Wq_f, Wk_f, Wv_f, Wo_f))
    ):
        s = DH ** -0.5
        w_np = {
            "wq": np.concatenate([(Wq_f * s).astype(BF)] * 8, axis=0),
            "wk": np.concatenate([Wk_f.astype(BF)] * 8, axis=0),
            "wv": np.concatenate([Wv_f.astype(BF)] * 8, axis=0),
            "wo": np.concatenate([Wo_f.astype(BF)] * 8, axis=0),
        }
        _CACHE["w_dev"] = {k: rt["device_put"](v, rt["shc"]) for k, v in w_np.items()}
        _CACHE["wkey"] = (Wq_f.copy(), Wk_f.copy(), Wv_f.copy(), Wo_f.copy())
    w_dev = _CACHE["w_dev"]

    B = x.shape[0]
    N = x.shape[2] * x.shape[3]
    xf = x.reshape(B, C, N)

    # y first: kv build is on the critical path of every q chunk
    yn_cat = rt["prep_y"](y, ln_y_g, ln_y_b)
    yd = rt["device_put"](np.asarray(yn_cat), rt["shc"])
    kv_args = {"yn": yd, "wk": w_dev["wk"], "wv": w_dev["wv"]}
    kt_d, v_d = rt["kv_sharded"](*[kv_args[n] for n in rt["kv_in"]],
                                 *rt["kv_dummy"])

    xn, xn_cat = rt["prep_x"](xf, ln_x_g, ln_x_b)
    xn_np = np.asarray(xn_cat)
    outs = []
    for k in range(K_CHUNKS):
        xd = rt["device_put"](xn_np[k], rt["shc"])
        q_args = {"xn": xd, "kt": kt_d, "v": v_d,
                  "wq": w_dev["wq"], "wo": w_dev["wo"]}
        (o,) = rt["q_sharded"](*[q_args[n] for n in rt["q_in"]], *rt["q_dummy"])
        outs.append(o)
        o.copy_to_host_async()

    oas = np.stack([np.asarray(o) for o in outs])    # [K, 8C, QCH] fp8
    cst = bv_f @ Wo_f + bo_f
    final = np.asarray(rt["post"](oas, xn, cst))     # [B, C, N] f32
    return final.swapaxes(1, 2)                      # [B, N, C] view


# revision 11
# speedup vs baseline: 6.6951x; 1.0401x over previous
"""CABlock cross-attention kernel for 8 TRN2 NeuronCores.

Sharding: 8 cores = 4 batches x 2 query-halves; each core computes an
independent output slice, no collectives.

The axon tunnel (~30-60MB/s each way, full-duplex) dominates wall time,
so the host/device split minimizes and overlaps bytes on the wire:
  - LayerNorm (incl. gains/biases) runs on host (jitted on the CPU
    backend); normalized activations ship as fp8 e4m3 in c-layout [C, N]
    (the natural layout of x, so no transposes anywhere).
  - Device work is split into a KV program (yn -> k,v, kept
    device-resident) and a Q program (xn chunk -> fp8 attention output
    chunk) dispatched K times; chunk uploads overlap earlier chunks'
    downloads on the duplex tunnel.
  - The residual add (exact f32 xn) and output biases (bv@Wo + bo)
    happen on host.
  - Weights (bf16) upload once and stay device-resident; jitted
    executables are cached; output operands are persistent device
    dummies (both programs write every output element, so no donated
    zero buffers ever cross the wire).
"""

import sys

import numpy as np

try:
    import concourse.bass as bass  # noqa: F401
except ImportError:
    sys.path.insert(0, "/opt/trn_rl_repo")
    import concourse.bass as bass

import ml_dtypes
import concourse.mybir as mybir
import concourse.tile as tile

F32 = mybir.dt.float32
BF16 = mybir.dt.bfloat16
F8 = mybir.dt.float8e4
BF = ml_dtypes.bfloat16
F8NP = ml_dtypes.float8_e4m3

# per-core problem dims
NQ = 2048   # query rows per core (16 tiles of 128)
M = 1024    # context rows (8 tiles of 128)
C = 256     # model dim (2 chunks of 128)
INNER = 512  # heads*dim_head (4 chunks of 128)
H = 8       # heads
DH = 64     # dim_head
NQT = NQ // 128   # 16
MT = M // 128     # 8
CC = C // 128     # 2
IC = INNER // 128  # 4
EPS = 1e-5

K_CHUNKS = 4
QCH = NQ // K_CHUNKS       # query cols per chunk per core
QCT = QCH // 128           # q tiles per chunk

_CACHE = {}


def _split_multiwaits(nc):
    """walrus allows only one sem-wait per ISA instruction; move extra waits
    onto same-engine NoOps inserted immediately before the instruction."""
    cnt = 0
    for f in nc.m.functions:
        for b in f.blocks:
            out = []
            for inst in b.instructions:
                si = inst.sync_info
                if si is not None and si.on_wait and len(si.on_wait) > 1:
                    waits = list(si.on_wait)
                    for w in waits[:-1]:
                        cnt += 1
                        nop = mybir.InstNoOp(
                            name=f"WSPLIT-{cnt}",
                            ins=[], outs=[],
                            engine=inst.engine,
                            sync_info=mybir.SyncInfo(on_wait=[w], on_update=[]),
                            bass_nofuse=True,
                        )
                        out.append(nop)
                    inst.sync_info = mybir.SyncInfo(
                        on_wait=[waits[-1]], on_update=list(si.on_update)
                    )
                out.append(inst)
            b.instructions = out
    return nc


def _build_kv_nc():
    """yn (fp8, c-layout) -> kT [INNER, M] bf16, v_aug [M, H*(DH+1)] bf16."""
    nc = bass.Bass()
    y_ext = nc.declare_dram_parameter("yn", [C, M], F8, isOutput=False)
    wk_ext = nc.declare_dram_parameter("wk", [C, INNER], BF16, isOutput=False)
    wv_ext = nc.declare_dram_parameter("wv", [C, INNER], BF16, isOutput=False)
    kt_ext = nc.declare_dram_parameter("kt", [INNER, M], BF16, isOutput=True)
    v_ext = nc.declare_dram_parameter("v", [M, H * (DH + 1)], BF16, isOutput=True)

    with tile.TileContext(nc) as tc:
        with (
            tc.tile_pool(name="sb", bufs=1) as sb,
            tc.tile_pool(name="ps", bufs=4, space="PSUM") as ps,
        ):
            wk_sb = sb.tile([128, CC, INNER], BF16)
            nc.gpsimd.dma_start(wk_sb, wk_ext.rearrange("(kc p) i -> p kc i", p=128))
            wv_sb = sb.tile([128, CC, INNER], BF16)
            nc.gpsimd.dma_start(wv_sb, wv_ext.rearrange("(kc p) i -> p kc i", p=128))
            ynT = sb.tile([128, CC, M], F8)
            nc.gpsimd.dma_start(ynT, y_ext.rearrange("(kc p) n -> p kc n", p=128))

            kt = sb.tile([128, IC, M], BF16)
            for ic in range(IC):
                for mc in range(M // 512):
                    pk = ps.tile([128, 512], F32, tag="ps")
                    for kc in range(CC):
                        nc.tensor.matmul(
                            pk, lhsT=wk_sb[:, kc, ic * 128:(ic + 1) * 128],
                            rhs=ynT[:, kc, mc * 512:(mc + 1) * 512],
                            start=(kc == 0), stop=(kc == CC - 1),
                        )
                    nc.vector.tensor_copy(out=kt[:, ic, mc * 512:(mc + 1) * 512], in_=pk)
            nc.gpsimd.dma_start(kt_ext.rearrange("(ic p) m -> p ic m", p=128), kt)

            v_sb = sb.tile([128, MT, H, DH + 1], BF16)
            nc.vector.memset(v_sb[:, :, :, DH:DH + 1], 1.0)
            for mt in range(MT):
                pv = ps.tile([128, 512], F32, tag="ps")
                for kc in range(CC):
                    nc.tensor.matmul(
                        pv, lhsT=ynT[:, kc, mt * 128:(mt + 1) * 128],
                        rhs=wv_sb[:, kc, :],
                        start=(kc == 0), stop=(kc == CC - 1),
                    )
                nc.vector.tensor_copy(
                    out=v_sb[:, mt, :, 0:DH],
                    in_=pv.rearrange("p (h e) -> p h e", h=H),
                )
            nc.gpsimd.dma_start(
                v_ext.rearrange("(mt p) (h e) -> p mt h e", p=128, h=H), v_sb
            )
    return _split_multiwaits(nc)


def _build_q_nc():
    """xn chunk (fp8, c-layout) + device-resident kT/v -> out chunk fp8."""
    nc = bass.Bass()
    x_ext = nc.declare_dram_parameter("xn", [C, QCH], F8, isOutput=False)
    kt_ext = nc.declare_dram_parameter("kt", [INNER, M], BF16, isOutput=False)
    v_ext = nc.declare_dram_parameter("v", [M, H * (DH + 1)], BF16, isOutput=False)
    wq_ext = nc.declare_dram_parameter("wq", [C, INNER], BF16, isOutput=False)
    wo_ext = nc.declare_dram_parameter("wo", [INNER, C], BF16, isOutput=False)
    out_ext = nc.declare_dram_parameter("out", [C, QCH], F8, isOutput=True)

    from concourse.masks import make_identity

    with tile.TileContext(nc) as tc:
        with (
            tc.tile_pool(name="sb", bufs=1) as sb,
            tc.tile_pool(name="probs", bufs=4) as probs_pool,
            tc.tile_pool(name="stats", bufs=4) as stats,
            tc.tile_pool(name="ps_big", bufs=2, space="PSUM") as ps_big,
            tc.tile_pool(name="ps_small", bufs=4, space="PSUM") as ps_small,
        ):
            ident_bf = sb.tile([128, 128], BF16)
            make_identity(nc, ident_bf)

            wq_sb = sb.tile([128, CC, INNER], BF16)
            nc.gpsimd.dma_start(wq_sb, wq_ext.rearrange("(kc p) i -> p kc i", p=128))
            wo_sb = sb.tile([128, IC, C], BF16)
            nc.gpsimd.dma_start(wo_sb, wo_ext.rearrange("(ic p) c -> p ic c", p=128))
            kt = sb.tile([128, IC, M], BF16)
            nc.gpsimd.dma_start(kt, kt_ext.rearrange("(ic p) m -> p ic m", p=128))
            v_sb = sb.tile([128, MT, H, DH + 1], BF16)
            nc.gpsimd.dma_start(
                v_sb, v_ext.rearrange("(mt p) (h e) -> p mt h e", p=128, h=H)
            )
            xnT = sb.tile([128, CC, QCH], F8)
            nc.gpsimd.dma_start(xnT, x_ext.rearrange("(kc p) n -> p kc n", p=128))

            QW = min(512, QCH)        # matmul free-dim tile width

            # qT[inner, qch]
            qt = sb.tile([128, IC, QCH], BF16)
            for ic in range(IC):
                for nqc in range(QCH // QW):
                    pq = ps_small.tile([128, 512], F32, tag="ps_sm")
                    for kc in range(CC):
                        nc.tensor.matmul(
                            pq[:, :QW], lhsT=wq_sb[:, kc, ic * 128:(ic + 1) * 128],
                            rhs=xnT[:, kc, nqc * QW:(nqc + 1) * QW],
                            start=(kc == 0), stop=(kc == CC - 1),
                        )
                    nc.vector.tensor_copy(out=qt[:, ic, nqc * QW:(nqc + 1) * QW],
                                          in_=pq[:, :QW])

            # attention, head pairs
            o_sb = sb.tile([128, QCT, IC, 128], BF16)  # o[nq, inner]
            for hp in range(H // 2):
                pT = []
                for hh in range(2):
                    pT.append(probs_pool.tile([128, MT, QCH], BF16,
                                              tag="probsT", name=f"probsT_{hp}_{hh}"))
                for mt in range(MT):
                    pe = []
                    for hh in range(2):
                        p_e = ps_big.tile([128, QCH], F32, tag="escore")
                        lhsT = kt[hh * 64:(hh + 1) * 64, hp, mt * 128:(mt + 1) * 128]
                        for n2 in range(QCH // QW):
                            nc.tensor.matmul(
                                p_e[:, n2 * QW:(n2 + 1) * QW],
                                lhsT=lhsT,
                                rhs=qt[hh * 64:(hh + 1) * 64, hp,
                                       n2 * QW:(n2 + 1) * QW],
                                start=True, stop=True,
                            )
                        pe.append(p_e)
                    for hh in range(2):
                        nc.scalar.activation(
                            out=pT[hh][:, mt, :],
                            in_=pe[hh],
                            func=mybir.ActivationFunctionType.Exp,
                        )
                for lq in range(QCT):
                    for hh in range(2):
                        h = hp * 2 + hh
                        po = ps_small.tile([128, 512], F32, tag="ps_sm")
                        for mt in range(MT):
                            nc.tensor.matmul(
                                po[:, :DH + 1],
                                lhsT=pT[hh][:, mt, lq * 128:(lq + 1) * 128],
                                rhs=v_sb[:, mt, h, :],
                                start=(mt == 0), stop=(mt == MT - 1),
                            )
                        rs = stats.tile([128, 1], F32, tag="rs")
                        nc.vector.reciprocal(out=rs, in_=po[:, DH:DH + 1])
                        nc.vector.tensor_scalar_mul(
                            out=o_sb[:, lq, h // 2, (h % 2) * DH:(h % 2) * DH + DH],
                            in0=po[:, 0:DH], scalar1=rs,
                        )

            # transpose o -> oT[inner, nq]
            oT = sb.tile([128, IC, QCH], BF16)
            for ic in range(IC):
                for nqt in range(QCT):
                    pt = ps_small.tile([128, 512], BF16, tag="ps_sm")
                    nc.tensor.transpose(pt[:, :128], o_sb[:, nqt, ic, :], ident_bf)
                    nc.vector.tensor_copy(out=oT[:, ic, nqt * 128:(nqt + 1) * 128],
                                          in_=pt[:, :128])

            # out-proj, c-layout (lhsT/rhs swapped => no output transpose)
            out_sb = sb.tile([128, CC, QCH], F8)
            for cc in range(CC):
                for nqc in range(QCH // QW):
                    pf = ps_small.tile([128, 512], F32, tag="ps_sm")
                    for ic in range(IC):
                        nc.tensor.matmul(
                            pf[:, :QW],
                            lhsT=wo_sb[:, ic, cc * 128:(cc + 1) * 128],
                            rhs=oT[:, ic, nqc * QW:(nqc + 1) * QW],
                            start=(ic == 0), stop=(ic == IC - 1),
                        )
                    nc.vector.tensor_copy(out=out_sb[:, cc, nqc * QW:(nqc + 1) * QW],
                                          in_=pf[:, :QW])
            nc.gpsimd.dma_start(out_ext.rearrange("(cc p) n -> p cc n", p=128), out_sb)
    return _split_multiwaits(nc)


def _make_sharded(nc, jax, shard_map, mesh, PartitionSpec):
    """Jitted SPMD executor for a Bass module; output operands are plain
    (non-donated) parameters, so persistent dummies can back them."""
    from concourse.bass2jax import _bass_exec_p, partition_id_tensor

    partition_name = nc.partition_id_tensor.name if nc.partition_id_tensor else None
    in_names, out_names, out_avals = [], [], []
    for alloc in nc.m.functions[0].allocations:
        if not isinstance(alloc, mybir.MemoryLocationSet):
            continue
        name = alloc.memorylocations[0].name
        if alloc.kind == "ExternalInput":
            if name != partition_name:
                in_names.append(name)
        elif alloc.kind == "ExternalOutput":
            out_names.append(name)
            out_avals.append(jax.core.ShapedArray(
                tuple(alloc.tensor_shape), mybir.dt.np(alloc.dtype)))
    n_params = len(in_names)
    n_outs = len(out_avals)
    in_names_full = in_names + out_names + ([partition_name] if partition_name else [])

    def _body(*args):
        operands = list(args)
        if partition_name is not None:
            operands.append(partition_id_tensor())
        outs = _bass_exec_p.bind(
            *operands,
            out_avals=tuple(out_avals),
            in_names=tuple(in_names_full),
            out_names=tuple(out_names),
            lowering_input_output_aliases=(),
            sim_require_finite=True,
            sim_require_nnan=True,
            nc=nc,
        )
        return tuple(outs)

    sharded = jax.jit(
        shard_map(_body, mesh=mesh,
                  in_specs=(PartitionSpec("core"),) * (n_params + n_outs),
                  out_specs=(PartitionSpec("core"),) * n_outs,
                  check_rep=False),
        keep_unused=True,
    )
    return sharded, in_names, [(tuple(a.shape), a.dtype) for a in out_avals]


def _get_runtime():
    if "q_sharded" in _CACHE:
        return _CACHE
    import jax
    import jax.numpy as jnp
    from jax.sharding import Mesh, PartitionSpec, NamedSharding
    try:
        from jax.experimental.shard_map import shard_map
    except ImportError:
        from jax import shard_map
    from concourse.bass2jax import install_neuronx_cc_hook

    install_neuronx_cc_hook()
    devices = jax.devices()[:8]
    mesh = Mesh(np.asarray(devices), ("core",))
    shc = NamedSharding(mesh, PartitionSpec("core"))

    kv_sharded, kv_in, kv_outspec = _make_sharded(
        _build_kv_nc(), jax, shard_map, mesh, PartitionSpec)
    q_sharded, q_in, q_outspec = _make_sharded(
        _build_q_nc(), jax, shard_map, mesh, PartitionSpec)

    # persistent device dummies backing the output operands
    def dev_zeros(spec):
        return jax.jit(
            lambda: tuple(jnp.zeros((8 * s[0], *s[1:]), d) for s, d in spec),
            out_shardings=(shc,) * len(spec))()
    kv_dummy = dev_zeros(kv_outspec)
    q_dummy = dev_zeros(q_outspec)
    jax.block_until_ready((kv_dummy, q_dummy))

    cpu = jax.devices("cpu")[0]
    F8J = jnp.float8_e4m3

    def _prep_x(xr, g, b):
        # [B, C, N] -> exact f32 LN (over C) and per-chunk fp8 concats
        mu = xr.mean(axis=1, keepdims=True)
        var = (xr * xr).mean(axis=1, keepdims=True) - mu * mu
        xn = (xr - mu) * jax.lax.rsqrt(var + EPS)
        xn = xn * g[:, None] + b[:, None]
        # (b, c, hf, k, i) -> (k, b*hf*c, i)
        cat = (xn.astype(F8J).reshape(4, C, 2, K_CHUNKS, QCH)
               .transpose(3, 0, 2, 1, 4).reshape(K_CHUNKS, 8 * C, QCH))
        return xn, cat

    def _prep_y(yr, g, b):
        mu = yr.mean(axis=-1, keepdims=True)
        var = (yr * yr).mean(axis=-1, keepdims=True) - mu * mu
        yn = (yr - mu) * jax.lax.rsqrt(var + EPS) * g + b
        ynt8 = yn.transpose(0, 2, 1).astype(F8J)
        return jnp.repeat(ynt8, 2, axis=0).reshape(8 * C, M)

    def _post(oas, xn, cst):
        # oas [K, 8C, QCH] fp8 -> [B, C, N] f32 + residual + biases
        o = (oas.reshape(K_CHUNKS, 4, 2, C, QCH).astype(jnp.float32)
             .transpose(1, 3, 2, 0, 4).reshape(4, C, 2 * NQ))
        return o + xn + cst[None, :, None]

    def _on_cpu(fn):
        jfn = jax.jit(fn)

        def run(*a):
            with jax.default_device(cpu):
                return jfn(*a)
        return run

    _CACHE.update(
        kv_sharded=kv_sharded, kv_in=kv_in, kv_dummy=kv_dummy,
        q_sharded=q_sharded, q_in=q_in, q_dummy=q_dummy,
        shc=shc, jax=jax, device_put=jax.device_put,
        prep_x=_on_cpu(_prep_x), prep_y=_on_cpu(_prep_y), post=_on_cpu(_post),
    )
    return _CACHE


def kernel(x, y, ln_x_g, ln_x_b, ln_y_g, ln_y_b, Wq, Wk, Wv, bv, Wo, bo, **kw):
    x = np.asarray(x, np.float32)
    y = np.asarray(y, np.float32)
    ln_x_g = np.asarray(ln_x_g, np.float32)
    ln_x_b = np.asarray(ln_x_b, np.float32)
    ln_y_g = np.asarray(ln_y_g, np.float32)
    ln_y_b = np.asarray(ln_y_b, np.float32)
    Wq_f = np.asarray(Wq, np.float32)
    Wk_f = np.asarray(Wk, np.float32)
    Wv_f = np.asarray(Wv, np.float32)
    Wo_f = np.asarray(Wo, np.float32)
    bv_f = np.asarray(bv, np.float32)
    bo_f = np.asarray(bo, np.float32)

    rt = _get_runtime()

    # device-resident weights (re-uploaded only when they change)
    wkey = _CACHE.get("wkey")
    if wkey is None or not all(
        np.array_equal(wa, wb) for wa, wb in zip(wkey, (Wq_f, Wk_f, Wv_f, Wo_f))
    ):
        s = DH ** -0.5
        w_np = {
            "wq": np.concatenate([(Wq_f * s).astype(BF)] * 8, axis=0),
            "wk": np.concatenate([Wk_f.astype(BF)] * 8, axis=0),
            "wv": np.concatenate([Wv_f.astype(BF)] * 8, axis=0),
            "wo": np.concatenate([Wo_f.astype(BF)] * 8, axis=0),
        }
        _CACHE["w_dev"] = {k: rt["device_put"](v, rt["shc"]) for k, v in w_np.items()}
        _CACHE["wkey"] = (Wq_f.copy(), Wk_f.copy(), Wv_f.copy(), Wo_f.copy())
    w_dev = _CACHE["w_dev"]

    B = x.shape[0]
    N = x.shape[2] * x.shape[3]
    xf = x.reshape(B, C, N)

    # y first: kv build is on the critical path of every q chunk
    yn_cat = rt["prep_y"](y, ln_y_g, ln_y_b)
    yd = rt["device_put"](np.asarray(yn_cat), rt["shc"])
    kv_args = {"yn": yd, "wk": w_dev["wk"], "wv": w_dev["wv"]}
    kt_d, v_d = rt["kv_sharded"](*[kv_args[n] for n in rt["kv_in"]],
                                 *rt["kv_dummy"])

    xn, xn_cat = rt["prep_x"](xf, ln_x_g, ln_x_b)
    xn_np = np.asarray(xn_cat)
    outs = []
    for k in range(K_CHUNKS):
        xd = rt["device_put"](xn_np[k], rt["shc"])
        q_args = {"xn": xd, "kt": kt_d, "v": v_d,
                  "wq": w_dev["wq"], "wo": w_dev["wo"]}
        (o,) = rt["q_sharded"](*[q_args[n] for n in rt["q_in"]], *rt["q_dummy"])
        outs.append(o)
        o.copy_to_host_async()

    # residual + biases in f32, fused into the per-chunk fetches so the
    # post-processing of chunk k overlaps chunk k+1's download
    resid = np.asarray(xn)                           # [B, C, N] f32, zero-copy
    cst = bv_f @ Wo_f + bo_f
    if cst.any():
        resid = resid + cst[None, :, None]
    final = np.empty((B, C, N), np.float32)
    fv = final.reshape(B, C, 2, K_CHUNKS, QCH)
    rv = resid.reshape(B, C, 2, K_CHUNKS, QCH)
    for k in range(K_CHUNKS):
        oa_k = np.asarray(outs[k]).reshape(B, 2, C, QCH)
        for hf in range(2):
            np.add(oa_k[:, hf].astype(np.float32), rv[:, :, hf, k],
                   out=fv[:, :, hf, k])
    return final.swapaxes(1, 2)                      # [B, N, C] view


# revision 12
# speedup vs baseline: 7.1196x; 1.0634x over previous
"""CABlock cross-attention kernel for 8 TRN2 NeuronCores.

Sharding: 8 cores = 4 batches x 2 query-halves; each core computes an
independent output slice, no collectives.

The axon tunnel (~30-60MB/s each way, full-duplex) dominates wall time,
so the host/device split minimizes and overlaps bytes on the wire:
  - LayerNorm (incl. gains/biases) runs on host (jitted on the CPU
    backend); normalized activations ship as fp8 e4m3 in c-layout [C, N]
    (the natural layout of x, so no transposes anywhere).
  - Device work is split into a KV program (yn -> k,v, kept
    device-resident) and a Q program (xn chunk -> fp8 attention output
    chunk) dispatched K times; chunk uploads overlap earlier chunks'
    downloads on the duplex tunnel.
  - The residual add (exact f32 xn) and output biases (bv@Wo + bo)
    happen on host.
  - Weights (bf16) upload once and stay device-resident; jitted
    executables are cached; output operands are persistent device
    dummies (both programs write every output element, so no donated
    zero buffers ever cross the wire).
"""

import sys

import numpy as np

try:
    import concourse.bass as bass  # noqa: F401
except ImportError:
    sys.path.insert(0, "/opt/trn_rl_repo")
    import concourse.bass as bass

import ml_dtypes
import concourse.mybir as mybir
import concourse.tile as tile

F32 = mybir.dt.float32
BF16 = mybir.dt.bfloat16
F8 = mybir.dt.float8e4
BF = ml_dtypes.bfloat16
F8NP = ml_dtypes.float8_e4m3

# per-core problem dims
NQ = 2048   # query rows per core (16 tiles of 128)
M = 1024    # context rows (8 tiles of 128)
C = 256     # model dim (2 chunks of 128)
INNER = 512  # heads*dim_head (4 chunks of 128)
H = 8       # heads
DH = 64     # dim_head
NQT = NQ // 128   # 16
MT = M // 128     # 8
CC = C // 128     # 2
IC = INNER // 128  # 4
EPS = 1e-5

K_CHUNKS = 8
QCH = NQ // K_CHUNKS       # query cols per chunk per core
QCT = QCH // 128           # q tiles per chunk

_CACHE = {}


def _split_multiwaits(nc):
    """walrus allows only one sem-wait per ISA instruction; move extra waits
    onto same-engine NoOps inserted immediately before the instruction."""
    cnt = 0
    for f in nc.m.functions:
        for b in f.blocks:
            out = []
            for inst in b.instructions:
                si = inst.sync_info
                if si is not None and si.on_wait and len(si.on_wait) > 1:
                    waits = list(si.on_wait)
                    for w in waits[:-1]:
                        cnt += 1
                        nop = mybir.InstNoOp(
                            name=f"WSPLIT-{cnt}",
                            ins=[], outs=[],
                            engine=inst.engine,
                            sync_info=mybir.SyncInfo(on_wait=[w], on_update=[]),
                            bass_nofuse=True,
                        )
                        out.append(nop)
                    inst.sync_info = mybir.SyncInfo(
                        on_wait=[waits[-1]], on_update=list(si.on_update)
                    )
                out.append(inst)
            b.instructions = out
    return nc


def _build_kv_nc():
    """yn (fp8, c-layout) -> kT [INNER, M] bf16, v_aug [M, H*(DH+1)] bf16."""
    nc = bass.Bass()
    y_ext = nc.declare_dram_parameter("yn", [C, M], F8, isOutput=False)
    wk_ext = nc.declare_dram_parameter("wk", [C, INNER], BF16, isOutput=False)
    wv_ext = nc.declare_dram_parameter("wv", [C, INNER], BF16, isOutput=False)
    kt_ext = nc.declare_dram_parameter("kt", [INNER, M], BF16, isOutput=True)
    v_ext = nc.declare_dram_parameter("v", [M, H * (DH + 1)], BF16, isOutput=True)

    with tile.TileContext(nc) as tc:
        with (
            tc.tile_pool(name="sb", bufs=1) as sb,
            tc.tile_pool(name="ps", bufs=4, space="PSUM") as ps,
        ):
            wk_sb = sb.tile([128, CC, INNER], BF16)
            nc.gpsimd.dma_start(wk_sb, wk_ext.rearrange("(kc p) i -> p kc i", p=128))
            wv_sb = sb.tile([128, CC, INNER], BF16)
            nc.gpsimd.dma_start(wv_sb, wv_ext.rearrange("(kc p) i -> p kc i", p=128))
            ynT = sb.tile([128, CC, M], F8)
            nc.gpsimd.dma_start(ynT, y_ext.rearrange("(kc p) n -> p kc n", p=128))

            kt = sb.tile([128, IC, M], BF16)
            for ic in range(IC):
                for mc in range(M // 512):
                    pk = ps.tile([128, 512], F32, tag="ps")
                    for kc in range(CC):
                        nc.tensor.matmul(
                            pk, lhsT=wk_sb[:, kc, ic * 128:(ic + 1) * 128],
                            rhs=ynT[:, kc, mc * 512:(mc + 1) * 512],
                            start=(kc == 0), stop=(kc == CC - 1),
                        )
                    nc.vector.tensor_copy(out=kt[:, ic, mc * 512:(mc + 1) * 512], in_=pk)
            nc.gpsimd.dma_start(kt_ext.rearrange("(ic p) m -> p ic m", p=128), kt)

            v_sb = sb.tile([128, MT, H, DH + 1], BF16)
            nc.vector.memset(v_sb[:, :, :, DH:DH + 1], 1.0)
            for mt in range(MT):
                pv = ps.tile([128, 512], F32, tag="ps")
                for kc in range(CC):
                    nc.tensor.matmul(
                        pv, lhsT=ynT[:, kc, mt * 128:(mt + 1) * 128],
                        rhs=wv_sb[:, kc, :],
                        start=(kc == 0), stop=(kc == CC - 1),
                    )
                nc.vector.tensor_copy(
                    out=v_sb[:, mt, :, 0:DH],
                    in_=pv.rearrange("p (h e) -> p h e", h=H),
                )
            nc.gpsimd.dma_start(
                v_ext.rearrange("(mt p) (h e) -> p mt h e", p=128, h=H), v_sb
            )
    return _split_multiwaits(nc)


def _build_q_nc():
    """xn chunk (fp8, c-layout) + device-resident kT/v -> out chunk fp8."""
    nc = bass.Bass()
    x_ext = nc.declare_dram_parameter("xn", [C, QCH], F8, isOutput=False)
    kt_ext = nc.declare_dram_parameter("kt", [INNER, M], BF16, isOutput=False)
    v_ext = nc.declare_dram_parameter("v", [M, H * (DH + 1)], BF16, isOutput=False)
    wq_ext = nc.declare_dram_parameter("wq", [C, INNER], BF16, isOutput=False)
    wo_ext = nc.declare_dram_parameter("wo", [INNER, C], BF16, isOutput=False)
    out_ext = nc.declare_dram_parameter("out", [C, QCH], F8, isOutput=True)

    from concourse.masks import make_identity

    with tile.TileContext(nc) as tc:
        with (
            tc.tile_pool(name="sb", bufs=1) as sb,
            tc.tile_pool(name="probs", bufs=4) as probs_pool,
            tc.tile_pool(name="stats", bufs=4) as stats,
            tc.tile_pool(name="ps_big", bufs=2, space="PSUM") as ps_big,
            tc.tile_pool(name="ps_small", bufs=4, space="PSUM") as ps_small,
        ):
            ident_bf = sb.tile([128, 128], BF16)
            make_identity(nc, ident_bf)

            wq_sb = sb.tile([128, CC, INNER], BF16)
            nc.gpsimd.dma_start(wq_sb, wq_ext.rearrange("(kc p) i -> p kc i", p=128))
            wo_sb = sb.tile([128, IC, C], BF16)
            nc.gpsimd.dma_start(wo_sb, wo_ext.rearrange("(ic p) c -> p ic c", p=128))
            kt = sb.tile([128, IC, M], BF16)
            nc.gpsimd.dma_start(kt, kt_ext.rearrange("(ic p) m -> p ic m", p=128))
            v_sb = sb.tile([128, MT, H, DH + 1], BF16)
            nc.gpsimd.dma_start(
                v_sb, v_ext.rearrange("(mt p) (h e) -> p mt h e", p=128, h=H)
            )
            xnT = sb.tile([128, CC, QCH], F8)
            nc.gpsimd.dma_start(xnT, x_ext.rearrange("(kc p) n -> p kc n", p=128))

            QW = min(512, QCH)        # matmul free-dim tile width

            # qT[inner, qch]
            qt = sb.tile([128, IC, QCH], BF16)
            for ic in range(IC):
                for nqc in range(QCH // QW):
                    pq = ps_small.tile([128, 512], F32, tag="ps_sm")
                    for kc in range(CC):
                        nc.tensor.matmul(
                            pq[:, :QW], lhsT=wq_sb[:, kc, ic * 128:(ic + 1) * 128],
                            rhs=xnT[:, kc, nqc * QW:(nqc + 1) * QW],
                            start=(kc == 0), stop=(kc == CC - 1),
                        )
                    nc.vector.tensor_copy(out=qt[:, ic, nqc * QW:(nqc + 1) * QW],
                                          in_=pq[:, :QW])

            # attention, head pairs
            o_sb = sb.tile([128, QCT, IC, 128], BF16)  # o[nq, inner]
            for hp in range(H // 2):
                pT = []
                for hh in range(2):
                    pT.append(probs_pool.tile([128, MT, QCH], BF16,
                                              tag="probsT", name=f"probsT_{hp}_{hh}"))
                for mt in range(MT):
                    pe = []
                    for hh in range(2):
                        p_e = ps_big.tile([128, QCH], F32, tag="escore")
                        lhsT = kt[hh * 64:(hh + 1) * 64, hp, mt * 128:(mt + 1) * 128]
                        for n2 in range(QCH // QW):
                            nc.tensor.matmul(
                                p_e[:, n2 * QW:(n2 + 1) * QW],
                                lhsT=lhsT,
                                rhs=qt[hh * 64:(hh + 1) * 64, hp,
                                       n2 * QW:(n2 + 1) * QW],
                                start=True, stop=True,
                            )
                        pe.append(p_e)
                    for hh in range(2):
                        nc.scalar.activation(
                            out=pT[hh][:, mt, :],
                            in_=pe[hh],
                            func=mybir.ActivationFunctionType.Exp,
                        )
                for lq in range(QCT):
                    for hh in range(2):
                        h = hp * 2 + hh
                        po = ps_small.tile([128, 512], F32, tag="ps_sm")
                        for mt in range(MT):
                            nc.tensor.matmul(
                                po[:, :DH + 1],
                                lhsT=pT[hh][:, mt, lq * 128:(lq + 1) * 128],
                                rhs=v_sb[:, mt, h, :],
                                start=(mt == 0), stop=(mt == MT - 1),
                            )
                        rs = stats.tile([128, 1], F32, tag="rs")
                        nc.vector.reciprocal(out=rs, in_=po[:, DH:DH + 1])
                        nc.vector.tensor_scalar_mul(
                            out=o_sb[:, lq, h // 2, (h % 2) * DH:(h % 2) * DH + DH],
                            in0=po[:, 0:DH], scalar1=rs,
                        )

            # transpose o -> oT[inner, nq]
            oT = sb.tile([128, IC, QCH], BF16)
            for ic in range(IC):
                for nqt in range(QCT):
                    pt = ps_small.tile([128, 512], BF16, tag="ps_sm")
                    nc.tensor.transpose(pt[:, :128], o_sb[:, nqt, ic, :], ident_bf)
                    nc.vector.tensor_copy(out=oT[:, ic, nqt * 128:(nqt + 1) * 128],
                                          in_=pt[:, :128])

            # out-proj, c-layout (lhsT/rhs swapped => no output transpose)
            out_sb = sb.tile([128, CC, QCH], F8)
            for cc in range(CC):
                for nqc in range(QCH // QW):
                    pf = ps_small.tile([128, 512], F32, tag="ps_sm")
                    for ic in range(IC):
                        nc.tensor.matmul(
                            pf[:, :QW],
                            lhsT=wo_sb[:, ic, cc * 128:(cc + 1) * 128],
                            rhs=oT[:, ic, nqc * QW:(nqc + 1) * QW],
                            start=(ic == 0), stop=(ic == IC - 1),
                        )
                    nc.vector.tensor_copy(out=out_sb[:, cc, nqc * QW:(nqc + 1) * QW],
                                          in_=pf[:, :QW])
            nc.gpsimd.dma_start(out_ext.rearrange("(cc p) n -> p cc n", p=128), out_sb)
    return _split_multiwaits(nc)


def _make_sharded(nc, jax, shard_map, mesh, PartitionSpec):
    """Jitted SPMD executor for a Bass module; output operands are plain
    (non-donated) parameters, so persistent dummies can back them."""
    from concourse.bass2jax import _bass_exec_p, partition_id_tensor

    partition_name = nc.partition_id_tensor.name if nc.partition_id_tensor else None
    in_names, out_names, out_avals = [], [], []
    for alloc in nc.m.functions[0].allocations:
        if not isinstance(alloc, mybir.MemoryLocationSet):
            continue
        name = alloc.memorylocations[0].name
        if alloc.kind == "ExternalInput":
            if name != partition_name:
                in_names.append(name)
        elif alloc.kind == "ExternalOutput":
            out_names.append(name)
            out_avals.append(jax.core.ShapedArray(
                tuple(alloc.tensor_shape), mybir.dt.np(alloc.dtype)))
    n_params = len(in_names)
    n_outs = len(out_avals)
    in_names_full = in_names + out_names + ([partition_name] if partition_name else [])

    def _body(*args):
        operands = list(args)
        if partition_name is not None:
            operands.append(partition_id_tensor())
        outs = _bass_exec_p.bind(
            *operands,
            out_avals=tuple(out_avals),
            in_names=tuple(in_names_full),
            out_names=tuple(out_names),
            lowering_input_output_aliases=(),
            sim_require_finite=True,
            sim_require_nnan=True,
            nc=nc,
        )
        return tuple(outs)

    sharded = jax.jit(
        shard_map(_body, mesh=mesh,
                  in_specs=(PartitionSpec("core"),) * (n_params + n_outs),
                  out_specs=(PartitionSpec("core"),) * n_outs,
                  check_rep=False),
        keep_unused=True,
    )
    return sharded, in_names, [(tuple(a.shape), a.dtype) for a in out_avals]


def _get_runtime():
    if "q_sharded" in _CACHE:
        return _CACHE
    import jax
    import jax.numpy as jnp
    from jax.sharding import Mesh, PartitionSpec, NamedSharding
    try:
        from jax.experimental.shard_map import shard_map
    except ImportError:
        from jax import shard_map
    from concourse.bass2jax import install_neuronx_cc_hook

    install_neuronx_cc_hook()
    devices = jax.devices()[:8]
    mesh = Mesh(np.asarray(devices), ("core",))
    shc = NamedSharding(mesh, PartitionSpec("core"))

    kv_sharded, kv_in, kv_outspec = _make_sharded(
        _build_kv_nc(), jax, shard_map, mesh, PartitionSpec)
    q_sharded, q_in, q_outspec = _make_sharded(
        _build_q_nc(), jax, shard_map, mesh, PartitionSpec)

    # persistent device dummies backing the output operands
    def dev_zeros(spec):
        return jax.jit(
            lambda: tuple(jnp.zeros((8 * s[0], *s[1:]), d) for s, d in spec),
            out_shardings=(shc,) * len(spec))()
    kv_dummy = dev_zeros(kv_outspec)
    q_dummy = dev_zeros(q_outspec)
    jax.block_until_ready((kv_dummy, q_dummy))

    cpu = jax.devices("cpu")[0]
    F8J = jnp.float8_e4m3

    def _prep_x(xr, g, b):
        # [B, C, N] -> exact f32 LN (over C) and per-chunk fp8 concats
        mu = xr.mean(axis=1, keepdims=True)
        var = (xr * xr).mean(axis=1, keepdims=True) - mu * mu
        xn = (xr - mu) * jax.lax.rsqrt(var + EPS)
        xn = xn * g[:, None] + b[:, None]
        # (b, c, hf, k, i) -> (k, b*hf*c, i)
        cat = (xn.astype(F8J).reshape(4, C, 2, K_CHUNKS, QCH)
               .transpose(3, 0, 2, 1, 4).reshape(K_CHUNKS, 8 * C, QCH))
        return xn, cat

    def _prep_y(yr, g, b):
        mu = yr.mean(axis=-1, keepdims=True)
        var = (yr * yr).mean(axis=-1, keepdims=True) - mu * mu
        yn = (yr - mu) * jax.lax.rsqrt(var + EPS) * g + b
        ynt8 = yn.transpose(0, 2, 1).astype(F8J)
        return jnp.repeat(ynt8, 2, axis=0).reshape(8 * C, M)

    def _post(oas, xn, cst):
        # oas [K, 8C, QCH] fp8 -> [B, C, N] f32 + residual + biases
        o = (oas.reshape(K_CHUNKS, 4, 2, C, QCH).astype(jnp.float32)
             .transpose(1, 3, 2, 0, 4).reshape(4, C, 2 * NQ))
        return o + xn + cst[None, :, None]

    def _on_cpu(fn):
        jfn = jax.jit(fn)

        def run(*a):
            with jax.default_device(cpu):
                return jfn(*a)
        return run

    _CACHE.update(
        kv_sharded=kv_sharded, kv_in=kv_in, kv_dummy=kv_dummy,
        q_sharded=q_sharded, q_in=q_in, q_dummy=q_dummy,
        shc=shc, jax=jax, device_put=jax.device_put,
        prep_x=_on_cpu(_prep_x), prep_y=_on_cpu(_prep_y), post=_on_cpu(_post),
    )
    return _CACHE


def kernel(x, y, ln_x_g, ln_x_b, ln_y_g, ln_y_b, Wq, Wk, Wv, bv, Wo, bo, **kw):
    x = np.asarray(x, np.float32)
    y = np.asarray(y, np.float32)
    ln_x_g = np.asarray(ln_x_g, np.float32)
    ln_x_b = np.asarray(ln_x_b, np.float32)
    ln_y_g = np.asarray(ln_y_g, np.float32)
    ln_y_b = np.asarray(ln_y_b, np.float32)
    Wq_f = np.asarray(Wq, np.float32)
    Wk_f = np.asarray(Wk, np.float32)
    Wv_f = np.asarray(Wv, np.float32)
    Wo_f = np.asarray(Wo, np.float32)
    bv_f = np.asarray(bv, np.float32)
    bo_f = np.asarray(bo, np.float32)

    rt = _get_runtime()

    # device-resident weights (re-uploaded only when they change)
    wkey = _CACHE.get("wkey")
    if wkey is None or not all(
        np.array_equal(wa, wb) for wa, wb in zip(wkey, (Wq_f, Wk_f, Wv_f, Wo_f))
    ):
        s = DH ** -0.5
        w_np = {
            "wq": np.concatenate([(Wq_f * s).astype(BF)] * 8, axis=0),
            "wk": np.concatenate([Wk_f.astype(BF)] * 8, axis=0),
            "wv": np.concatenate([Wv_f.astype(BF)] * 8, axis=0),
            "wo": np.concatenate([Wo_f.astype(BF)] * 8, axis=0),
        }
        _CACHE["w_dev"] = {k: rt["device_put"](v, rt["shc"]) for k, v in w_np.items()}
        _CACHE["wkey"] = (Wq_f.copy(), Wk_f.copy(), Wv_f.copy(), Wo_f.copy())
    w_dev = _CACHE["w_dev"]

    B = x.shape[0]
    N = x.shape[2] * x.shape[3]
    xf = x.reshape(B, C, N)

    # y first: kv build is on the critical path of every q chunk
    yn_cat = rt["prep_y"](y, ln_y_g, ln_y_b)
    yd = rt["device_put"](np.asarray(yn_cat), rt["shc"])
    kv_args = {"yn": yd, "wk": w_dev["wk"], "wv": w_dev["wv"]}
    kt_d, v_d = rt["kv_sharded"](*[kv_args[n] for n in rt["kv_in"]],
                                 *rt["kv_dummy"])

    xn, xn_cat = rt["prep_x"](xf, ln_x_g, ln_x_b)
    xn_np = np.asarray(xn_cat)
    outs = []
    for k in range(K_CHUNKS):
        xd = rt["device_put"](xn_np[k], rt["shc"])
        q_args = {"xn": xd, "kt": kt_d, "v": v_d,
                  "wq": w_dev["wq"], "wo": w_dev["wo"]}
        (o,) = rt["q_sharded"](*[q_args[n] for n in rt["q_in"]], *rt["q_dummy"])
        outs.append(o)
        o.copy_to_host_async()

    # residual + biases in f32, fused into the per-chunk fetches so the
    # post-processing of chunk k overlaps chunk k+1's download
    resid = np.asarray(xn)                           # [B, C, N] f32, zero-copy
    cst = bv_f @ Wo_f + bo_f
    if cst.any():
        resid = resid + cst[None, :, None]
    final = np.empty((B, C, N), np.float32)
    fv = final.reshape(B, C, 2, K_CHUNKS, QCH)
    rv = resid.reshape(B, C, 2, K_CHUNKS, QCH)
    for k in range(K_CHUNKS):
        oa_k = np.asarray(outs[k]).reshape(B, 2, C, QCH)
        for hf in range(2):
            np.add(oa_k[:, hf].astype(np.float32), rv[:, :, hf, k],
                   out=fv[:, :, hf, k])
    return final.swapaxes(1, 2)                      # [B, N, C] view
